# revision 1
# baseline (speedup 1.0000x reference)
"""Bass/Tile TRN2 kernel for a non-local attention block (BaseNonLocalBlock).

Contract: kernel(**inputs) takes the FULL inputs of the nn.Module problem
(B=1, D=256, H=4, N=4096) and returns the FULL output [1, 256, 4096].

Sharding: query columns of the N x N attention are split across the 8
NeuronCores (512 queries per core). K/V projections are computed
redundantly on every core (cheap); each core produces its own output
column slice and the host concatenates.

Per-core structure (flash-attention style, scores never hit HBM):
  pre-phase: Q/K/V conv1x1 projections as fp8 DoubleRow matmuls
    (channels packed planar [128, 2, *]; weights prescaled x16 on the
    host, un-scaled for free in the PSUM->SBUF copy scale).  K -> bf16
    per-block tiles; V^T(+ones col per head) -> fp8 per-block tiles.
    Startup DMAs are spread over the sync/scalar/gpsimd rings (one
    HWDGE queue sustains only ~60 GB/s); a ~4us block of dummy matmuls
    warms the PE HAM clock gate (1.2 -> 2.4 GHz) during the DMA ramp.
  loop over 32 key chunks (128 keys each):
    S_T = K_h[:, chunk]^T @ Q_h     (PSUM, 2 heads per row-split pair)
    el  = spt * S_T                 (DVE mult - THE pace-setting op:
                                     PSUM fp32 reads are 1 elem/cycle/
                                     lane, ~2.3us per iteration)
    e2  = exp(el)                   (one fused ACT exp per iter -> fp8)
    msg_h += vt^T @ e2              (fp8 matmul per head, 1 iter behind;
                                     vt row 64 of ones accumulates the
                                     softmax denominator for free)
  tail (pipelined over two query halves): denominators -> PE ones-
    broadcast -> reciprocal_approx_fast -> per-head multiply -> conv
    MLP with BN folded into W1/W2 -> residual add (fp32 x copy).

Scores/messages fit in the PE slack under the DVE pace; exp fits ACT.
keep_warm() dummy matmuls paper over PE idle gaps so the HAM activity
monitor never re-throttles the clock mid-kernel.  fp8 numerics hold
the end-to-end rel error at ~1.6e-4 (tolerance 2e-2): exp arguments
|s*a| < 4.1 stay within fp8e4 range (max 240), and softmax weight
quantization error largely cancels between numerator and denominator.
"""
import numpy as np
from contextlib import ExitStack

D = 256
N = 4096
NQ = 512          # queries per core
H = 4
DH = 64
NCORES = 8
NIT = N // 128    # 32 key chunks
NPAIR = NIT // 2  # 16 chunk pairs (fp8 DoubleRow message matmuls)
VTS = 68          # padded per-head stride in the V_T-aug tile
WS = 16.0         # host prescale on conv weights before fp8 quantization

_CACHE = {}


def _build(has_bq, has_bk, has_bv, has_b3):
    import concourse.bass as bass
    import concourse.tile as tile
    from concourse import bacc, mybir

    F32 = mybir.dt.float32
    BF16 = mybir.dt.bfloat16
    FP8 = mybir.dt.float8e4
    Id = mybir.ActivationFunctionType.Identity
    Exp = mybir.ActivationFunctionType.Exp
    Relu = mybir.ActivationFunctionType.Relu
    DR = mybir.MatmulPerfMode.DoubleRow

    nc = bacc.Bacc("TRN2", target_bir_lowering=False, debug=False,
                   num_devices=NCORES)

    # DRAM I/O (per core)
    x8_d = nc.dram_tensor("x8", [4, 128, 2, N // 4], FP8,
                          kind="ExternalInput").ap()
    xq8_d = nc.dram_tensor("xq8", [128, 2, NQ], FP8, kind="ExternalInput").ap()
    xqr_d = nc.dram_tensor("xqr", [D, NQ], F32, kind="ExternalInput").ap()
    spt_d = nc.dram_tensor("spt", [N, NQ], BF16, kind="ExternalInput").ap()
    wq8_d = nc.dram_tensor("wq8", [128, 2, D], FP8, kind="ExternalInput").ap()
    wk8_d = nc.dram_tensor("wk8", [128, 2, D], FP8, kind="ExternalInput").ap()
    wv8_d = nc.dram_tensor("wv8", [128, 2, D], FP8, kind="ExternalInput").ap()
    w1t_d = nc.dram_tensor("w1t", [D, 128], BF16, kind="ExternalInput").ap()
    w2t_d = nc.dram_tensor("w2t", [128, 128], BF16, kind="ExternalInput").ap()
    w3t_d = nc.dram_tensor("w3t", [128, D], BF16, kind="ExternalInput").ap()
    bq_d = nc.dram_tensor("bq2", [128, 2], F32, kind="ExternalInput").ap()
    bk_d = nc.dram_tensor("bk2", [128, 2], F32, kind="ExternalInput").ap()
    bv_d = nc.dram_tensor("bv2", [128, 2], F32, kind="ExternalInput").ap()
    b1_d = nc.dram_tensor("b1f", [128, 1], F32, kind="ExternalInput").ap()
    b2_d = nc.dram_tensor("b2f", [128, 1], F32, kind="ExternalInput").ap()
    b3_d = nc.dram_tensor("b32", [128, 2], F32, kind="ExternalInput").ap()
    out_d = nc.dram_tensor("out", [D, NQ], F32, kind="ExternalOutput").ap()

    spt_t3 = spt_d.rearrange("(t p) o -> t p o", p=128)

    with tile.TileContext(nc) as tc, ExitStack() as ctx:
        sb = ctx.enter_context(tc.tile_pool(name="sb", bufs=1))
        spt_pool = ctx.enter_context(tc.tile_pool(name="sptp", bufs=10))
        el_pool = ctx.enter_context(tc.tile_pool(name="elp", bufs=4))
        e2_pool = ctx.enter_context(tc.tile_pool(name="e2p", bufs=4))
        pj_ctx = ExitStack()
        pj = pj_ctx.enter_context(tc.tile_pool(name="pj", bufs=4, space="PSUM"))

        # ---- early inputs. One HWDGE queue sustains only ~60 GB/s, so the
        # startup transfers are spread over the three rings: sync takes the
        # x quarters, scalar the small weight tensors plus half of quarter 0,
        # gpsimd the other half before its spt stream. ----
        x8t = [sb.tile([128, 2, 1024], FP8, name=f"x8_{k}") for k in range(4)]
        for k in (0, 1, 2):
            nc.sync.dma_start(x8t[k][:, 0, :], x8_d[k][:, 0, :])
        nc.sync.dma_start(x8t[2][:, 1, :], x8_d[2][:, 1, :])
        nc.sync.dma_start(x8t[3][:, 0, :], x8_d[3][:, 0, :])
        for k in (0, 1, 3):
            nc.gpsimd.dma_start(x8t[k][:, 1, :], x8_d[k][:, 1, :])
        wq8 = sb.tile([128, 2, D], FP8, name="wq8")
        wk8 = sb.tile([128, 2, D], FP8, name="wk8")
        wv8 = sb.tile([128, 2, D], FP8, name="wv8")
        nc.scalar.dma_start(wk8[:], wk8_d[:, :, :])
        nc.scalar.dma_start(wv8[:], wv8_d[:, :, :])
        xq8 = sb.tile([128, 2, NQ], FP8, name="xq8")
        nc.scalar.dma_start(xq8[:], xq8_d[:, :, :])
        nc.scalar.dma_start(wq8[:], wq8_d[:, :, :])
        if has_bq:
            bq = sb.tile([128, 2], F32, name="bq")
            nc.scalar.dma_start(bq[:], bq_d[:, :])
        if has_bk:
            bk = sb.tile([128, 2], F32, name="bk")
            nc.scalar.dma_start(bk[:], bk_d[:, :])

        k_sb = [sb.tile([128, 2, NQ], BF16, name=f"ksb{ib}") for ib in range(8)]
        q_sb = [sb.tile([128, NQ], BF16, name=f"q{co}") for co in range(2)]
        # V^T augmented: per key-chunk it, per head h: [64 V cols | ones | pad]
        vt = [sb.tile([128, 4, H, VTS], FP8, name=f"vt{ib}") for ib in range(8)]
        for ib in range(8):
            nc.gpsimd.memset(vt[ib][:, :, :, 64:65], 1.0)
        ones64 = sb.tile([1, 64], BF16, name="ones64")
        nc.gpsimd.memset(ones64[:], 1.0)

        # ---- PE warmup: ~3.5us of tiny matmuls during the DMA ramp so the
        # HAM clock gate is already at 8/8 when real projections start ----
        warm = sb.tile([128, NQ], BF16, name="warm")
        nc.vector.memset(warm[:].bitcast(F32)[:, 0:256], 0.0)
        wps = pj.tile([128, NQ], F32, tag="t")
        for r in range(12):
            nc.tensor.matmul(wps[0:64, :], warm[:, 0:64], warm[:],
                             start=True, stop=True)

        def keep_warm(ap, n):
            # dummy matmuls into a PSUM region that a later start=True matmul
            # fully overwrites; fills PE idle gaps so the HAM clock stays 8/8
            for r in range(n):
                nc.tensor.matmul(ap, warm[:, 0:64], warm[:, 0:64],
                                 start=True, stop=True)

        # spt prefetch on the (otherwise idle) GPSIMD DMA ring
        spt_tiles = {}

        def load_spt(it):
            # alternate rings: the loop-steady spt stream (2.1us/chunk at
            # ~60GB/s/queue) nearly saturates one ring and jitters the DVE;
            # odd chunks ride the sync ring, which is idle during the loop
            t = spt_pool.tile([128, NQ], BF16, tag="spt")
            if it % 2 == 0:
                nc.gpsimd.dma_start(t[:], spt_t3[it])
            else:
                nc.sync.dma_start(t[:], spt_t3[it])
            spt_tiles[it] = t

        for it in range(6):
            load_spt(it)

        def q_proj():
            # fp8 DoubleRow conv1x1, contraction = 256 channels
            for co in range(2):
                ps = pj.tile([128, NQ], F32, tag="t")
                nc.tensor.matmul(ps[:], wq8[:, :, co * 128:(co + 1) * 128],
                                 xq8[:], start=True, stop=True, perf_mode=DR)
                if has_bq:
                    nc.scalar.activation(q_sb[co][:], ps[:], Id,
                                         scale=1.0 / (WS * 8.0),
                                         bias=bq[:, co:co + 1])
                else:
                    nc.scalar.activation(q_sb[co][:], ps[:], Id,
                                         scale=1.0 / (WS * 8.0))

        # ---- K / V^T projections per 512-key block, copies chase on
        # alternating ACT/DVE ----
        cp = [0]

        def copy_scaled(dst, src, bias=None):
            if bias is not None:
                nc.scalar.activation(dst, src, Id, scale=1.0 / WS, bias=bias)
            elif cp[0] % 2 == 0:
                nc.scalar.activation(dst, src, Id, scale=1.0 / WS)
            else:
                nc.vector.tensor_scalar_mul(dst, src, 1.0 / WS)
            cp[0] += 1

        for ib in range(8):
            xt = x8t[ib // 2]
            xo = (ib % 2) * 512
            kps = pj.tile([128, 2, NQ], F32, tag="t")
            for co in range(2):
                nc.tensor.matmul(kps[:, co, :],
                                 wk8[:, :, co * 128:(co + 1) * 128],
                                 xt[:, :, xo:xo + 512],
                                 start=True, stop=True, perf_mode=DR)
            ksl = k_sb[ib][:]
            if has_bk:
                for co in range(2):
                    nc.scalar.activation(ksl[:, co, :], kps[:, co, :], Id,
                                         scale=1.0 / WS, bias=bk[:, co:co + 1])
            else:
                copy_scaled(ksl, kps[:])
            vps = pj.tile([128, 4, 256], F32, tag="t")
            for u in range(4):
                ko = xo + u * 128
                nc.tensor.matmul(vps[:, u, 0:D], xt[:, :, ko:ko + 128],
                                 wv8[:], start=True, stop=True,
                                 perf_mode=DR)
            vdst = vt[ib][:, 0:4, :, 0:64]
            vsrc = vps[:, 0:4, 0:D].rearrange("p w (h c) -> p w h c", h=H)
            copy_scaled(vdst, vsrc)
            if ib == 1:
                q_proj()
        wps2 = pj.tile([128, NQ], F32, tag="t")
        keep_warm(wps2[0:64, 0:64], 8)

        pj_ctx.close()
        ps_m = ctx.enter_context(tc.tile_pool(name="psm", bufs=1, space="PSUM"))
        pst_ctx = ExitStack()
        ps_t = pst_ctx.enter_context(
            tc.tile_pool(name="pst", bufs=2, space="PSUM"))
        # messages for all 4 heads in one 4-bank PSUM tensor; row 64 of each
        # bank accumulates the softmax denominator (ones column in vt)
        mps = ps_m.tile([65, H, NQ], F32, name="mps")

        # ---- attention loop ----
        pendq = []

        def emit_msgs(p):
            pit, e2p = p
            for h in range(H):
                nc.tensor.matmul(mps[:, h, :], vt[pit // 4][:, pit % 4, h, 0:65],
                                 e2p[:, h, :],
                                 start=(pit == 0), stop=(pit == NIT - 1),
                                 skip_group_check=True)

        for it in range(NIT):
            if it + 6 < NIT:
                load_spt(it + 6)
            spt_t = spt_tiles.pop(it)
            # broadcast the mask over the head pair (free-dim 0-stride)
            spt_b = bass.AP(tensor=spt_t.tensor, offset=spt_t.offset,
                            ap=[list(spt_t.ap[0]), [0, 2],
                                list(spt_t.ap[1])])
            e2 = e2_pool.tile([128, H, NQ], FP8, tag="e2")
            el = el_pool.tile([128, H, NQ], BF16, tag="el")
            sps_l = []
            for hp in range(2):
                sps = ps_t.tile([128, 2, NQ], F32, tag="t")
                sps_l.append(sps)
                if it < 2:
                    keep_warm(sps[0:64, 0, 0:64], 8)
                elif it < 4:
                    keep_warm(sps[0:64, 0, 0:64], 3)
                for jj in range(2):
                    ro = jj * 64
                    nc.tensor.matmul(
                        sps[:, jj, :],
                        k_sb[it // 4][ro:ro + 64, hp,
                                      (it % 4) * 128:(it % 4) * 128 + 128],
                        q_sb[hp][ro:ro + 64, :],
                        start=True, stop=True)
                if it < NIT - 1:
                    nc.vector.tensor_mul(el[:, 2 * hp:2 * hp + 2, :], sps[:],
                                         spt_b)
            if it == NIT - 1:
                # final iteration: pipeline the drain over two query halves so
                # exp/messages of half a overlap the half-b mask-multiply
                for p in pendq:
                    emit_msgs(p)
                pendq = []
                for qh in range(2):
                    ql = slice(qh * (NQ // 2), (qh + 1) * (NQ // 2))
                    sptq = spt_t[:, ql]
                    spt_bq = bass.AP(tensor=sptq.tensor, offset=sptq.offset,
                                     ap=[list(sptq.ap[0]), [0, 2],
                                         list(sptq.ap[1])])
                    for hp in range(2):
                        nc.vector.tensor_mul(el[:, 2 * hp:2 * hp + 2, ql],
                                             sps_l[hp][:, :, ql], spt_bq)
                    nc.scalar.activation(e2[:, :, ql], el[:, :, ql], Exp)
                    for h in range(H):
                        nc.tensor.matmul(mps[:, h, ql],
                                         vt[7][:, 3, h, 0:65], e2[:, h, ql],
                                         start=False, stop=True,
                                         skip_group_check=True)
            if it == 1:
                # prime the clock gate: iteration 1's message matmuls wait
                # ~2us on exp(0) (pipeline not yet primed); fill the PE idle
                # with dummies into the mps region that the first start=True
                # message matmul fully overwrites
                keep_warm(mps[0:64, 0, 0:64], 14)
            if it < NIT - 1:
                # messages run TWO iterations behind: the PE reaches them
                # ~850ns into an iteration but exp(it) completes ~4.3us after
                # the iteration starts, so a 1-deep lag stalls the PE (and the
                # following scores) every other iteration
                if len(pendq) >= 2:
                    emit_msgs(pendq.pop(0))
                nc.scalar.activation(e2[:], el[:], Exp)
                pendq.append((it, e2))

        # ---- late inputs (only needed after the attention loop) ----
        w1t = [sb.tile([128, 128], BF16, name=f"w1t{ci}") for ci in range(2)]
        for ci in range(2):
            nc.sync.dma_start(w1t[ci][:], w1t_d[ci * 128:(ci + 1) * 128, :])
        w2t = sb.tile([128, 128], BF16, name="w2t")
        nc.sync.dma_start(w2t[:], w2t_d[:, :])
        w3t = sb.tile([128, D], BF16, name="w3t")
        nc.sync.dma_start(w3t[:], w3t_d[:, :])
        xqr = [sb.tile([128, NQ], F32, name=f"xqr{co}") for co in range(2)]
        for co in range(2):
            nc.sync.dma_start(xqr[co][:], xqr_d[co * 128:(co + 1) * 128, :])
        b1 = sb.tile([128, 1], F32, name="b1")
        b2 = sb.tile([128, 1], F32, name="b2")
        nc.sync.dma_start(b1[:], b1_d[:, :])
        nc.sync.dma_start(b2[:], b2_d[:, :])
        if has_bv:
            bv = sb.tile([128, 2], F32, name="bv")
            nc.sync.dma_start(bv[:], bv_d[:, :])
        if has_b3:
            b3 = sb.tile([128, 2], F32, name="b3")
            nc.sync.dma_start(b3[:], b3_d[:, :])

        pst_ctx.close()
        pt = ctx.enter_context(tc.tile_pool(name="pt", bufs=4, space="PSUM"))

        # ---- tail, pipelined over two query halves: denominators -> PE
        # ones-broadcast -> reciprocal_approx_fast -> per-head multiply ->
        # conv MLP with BN folded -> residual add.  Halving the q extent
        # doubles pipeline overlap across PE/ACT/DVE for the serial chain. ----
        HQ = NQ // 2
        dhs = sb.tile([1, H, NQ], BF16, name="dhs")
        msg = [sb.tile([128, NQ], BF16, name=f"msg{co}") for co in range(2)]
        rbc = sb.tile([64, 2, 2, NQ], F32, name="rbc")
        h1 = sb.tile([128, NQ], BF16, name="h1")
        h2 = sb.tile([128, NQ], BF16, name="h2")
        ot = [sb.tile([128, NQ], F32, name=f"ot{co}") for co in range(2)]
        tb = sb.tile([128, NQ], F32, name="tb")
        for qh in range(2):
            ql = slice(qh * HQ, (qh + 1) * HQ)
            nc.scalar.copy(dhs[:, 0:2, ql], mps[64:65, 0:2, ql])
            nc.vector.tensor_copy(dhs[:, 2:4, ql], mps[64:65, 2:4, ql])
            for co in range(2):
                dbb = pt.tile([64, 2, HQ], F32, tag="t")
                for jj in range(2):
                    nc.tensor.matmul(dbb[:, jj, :], ones64[:],
                                     dhs[:, 2 * co + jj, ql],
                                     start=True, stop=True)
                nc.vector.reciprocal_approx_fast(out=rbc[:, co, :, ql],
                                                 in_=dbb[:])
                for jj in range(2):
                    h = 2 * co + jj
                    ro = jj * 64
                    nc.vector.tensor_mul(msg[co][ro:ro + 64, ql],
                                         mps[0:64, h, ql],
                                         rbc[:, co, jj, ql])
                    if has_bv:
                        nc.scalar.activation(msg[co][ro:ro + 64, ql],
                                             msg[co][ro:ro + 64, ql], Id,
                                             bias=bv[ro:ro + 64, co:co + 1])
            u1 = pt.tile([128, HQ], F32, tag="t")
            for ci in range(2):
                nc.tensor.matmul(u1[:], w1t[ci][:], msg[ci][:, ql],
                                 start=(ci == 0), stop=(ci == 1))
            nc.scalar.activation(h1[:, ql], u1[:], Relu, bias=b1[:, 0:1])
            u2 = pt.tile([128, HQ], F32, tag="t")
            nc.tensor.matmul(u2[:], w2t[:], h1[:, ql], start=True, stop=True)
            nc.scalar.activation(h2[:, ql], u2[:], Relu, bias=b2[:, 0:1])
            for co in range(2):
                u3 = pt.tile([128, HQ], F32, tag="t")
                nc.tensor.matmul(u3[:], w3t[:, co * 128:(co + 1) * 128],
                                 h2[:, ql], start=True, stop=True)
                if has_b3:
                    nc.scalar.activation(tb[:, ql], u3[:], Id,
                                         bias=b3[:, co:co + 1])
                    nc.vector.tensor_add(ot[co][:, ql], tb[:, ql],
                                         xqr[co][:, ql])
                else:
                    nc.vector.tensor_add(ot[co][:, ql], u3[:], xqr[co][:, ql])
                nc.sync.dma_start(out_d[co * 128:(co + 1) * 128, ql],
                                  ot[co][:, ql])

    nc.compile()
    return nc


def _prep_inputs(inputs):
    import ml_dtypes
    E4 = ml_dtypes.float8_e4m3
    bf = lambda a: np.ascontiguousarray(
        np.asarray(a, dtype=np.float32).astype(ml_dtypes.bfloat16))
    f8 = lambda a: np.ascontiguousarray(
        np.asarray(a, dtype=np.float32).astype(E4))
    f = lambda a: np.ascontiguousarray(np.asarray(a, dtype=np.float32))
    planar = lambda a: np.ascontiguousarray(
        np.asarray(a, np.float32).reshape(2, 128, -1).transpose(1, 0, 2))

    x32 = f(inputs["corr_feat_belief"][0])                  # [D, N]
    spT = np.asarray(inputs["spatial_compatibility"][0]).T  # [N(keys), N(q)]
    Wq, bq = f(inputs["Wq"]), f(inputs["bq"])
    Wk, bk = f(inputs["Wk"]), f(inputs["bk"])
    Wv, bv = f(inputs["Wv"]), f(inputs["bv"])
    W1, b1, g1, be1 = f(inputs["W1"]), f(inputs["b1"]), f(inputs["g1"]), f(inputs["be1"])
    W2, b2, g2, be2 = f(inputs["W2"]), f(inputs["b2"]), f(inputs["g2"]), f(inputs["be2"])
    W3, b3 = f(inputs["W3"]), f(inputs["b3"])

    scale = np.float32(1.0 / np.sqrt(DH))
    s1 = (g1 / np.sqrt(np.float32(1.0) + np.float32(1e-5))).astype(np.float32)
    s2 = (g2 / np.sqrt(np.float32(1.0) + np.float32(1e-5))).astype(np.float32)

    xpl = planar(x32)               # [128, 2, N]; channel c = p + 128*j
    x8 = f8(xpl)
    # quarter-major so each quarter DMA reads contiguous 2KB/partition lines
    x8q = np.ascontiguousarray(
        np.stack([x8[:, :, k * 1024:(k + 1) * 1024] for k in range(4)]))
    spT_bf = bf(spT)
    common = dict(
        x8=x8q,
        wq8=f8(planar(Wq.T) * WS),
        wk8=f8(planar(Wk.T) * WS),
        wv8=f8(planar(Wv.T) * WS),
        w1t=bf((W1 * s1[:, None]).T),
        w2t=bf((W2 * s2[:, None]).T),
        w3t=bf(W3.T),
        bq2=f((bq * scale).reshape(2, 128).T),
        bk2=f(bk.reshape(2, 128).T),
        bv2=f(bv.reshape(2, 128).T),
        b1f=f((s1 * b1 + be1).reshape(128, 1)),
        b2f=f((s2 * b2 + be2).reshape(128, 1)),
        b32=f(b3.reshape(2, 128).T),
    )
    in_maps = []
    for m in range(NCORES):
        sl = slice(m * NQ, (m + 1) * NQ)
        im = dict(common)
        im["xq8"] = np.ascontiguousarray(x8[:, :, sl])
        im["xqr"] = f(x32[:, sl])
        im["spt"] = np.ascontiguousarray(spT_bf[:, sl])
        in_maps.append(im)
    flags = tuple(bool(np.any(b != 0)) for b in (bq, bk, bv, b3))
    return in_maps, flags


def _run(inputs, trace=False):
    from concourse.bass_utils import run_bass_kernel_spmd
    in_maps, flags = _prep_inputs(inputs)
    if flags not in _CACHE:
        _CACHE[flags] = _build(*flags)
    nc = _CACHE[flags]
    res = run_bass_kernel_spmd(nc, in_maps, core_ids=list(range(NCORES)),
                               trace=trace)
    out = np.concatenate([res.results[m]["out"] for m in range(NCORES)],
                         axis=1)[None]
    return np.ascontiguousarray(out.astype(np.float32)), res


def kernel(**inputs):
    out, _ = _run(inputs, trace=False)
    return out



# revision 9
# speedup vs baseline: 1.0211x; 1.0211x over previous
"""Bass/Tile TRN2 kernel for a non-local attention block (BaseNonLocalBlock).

Contract: kernel(**inputs) takes the FULL inputs of the nn.Module problem
(B=1, D=256, H=4, N=4096) and returns the FULL output [1, 256, 4096].

Sharding: query columns of the N x N attention are split across the 8
NeuronCores (512 queries per core). K/V projections are computed
redundantly on every core (cheap); each core produces its own output
column slice and the host concatenates.

Per-core structure (flash-attention style, scores never hit HBM):
  pre-phase: Q/K/V conv1x1 projections as fp8 DoubleRow matmuls
    (channels packed planar [128, 2, *]; weights prescaled x16 on the
    host, un-scaled for free in the PSUM->SBUF copy scale).  K -> bf16
    per-block tiles; V^T(+ones col per head) -> bf16 per-block tiles.
    Startup DMAs are spread over the sync/scalar/gpsimd rings (one
    HWDGE queue sustains only ~60 GB/s); a ~4us block of dummy matmuls
    warms the PE HAM clock gate (1.2 -> 2.4 GHz) during the DMA ramp.
  loop over 32 key chunks (128 keys each):
    S_T = K_h[:, chunk]^T @ Q_h       (PSUM, 2 heads per row-split pair)
    sc  = copy(S_T)                   (ACT PSUM->SBUF bf16, 2x ~1.0us --
                                       ACT's only loop duty; EXP is gone)
    el  = spt * sc                    (DVE bf16 tensor_tensor at 2x mode,
                                       ~1.25us for all 4 heads)
    e2i = int16(el*184.665 + 16250.4) (DVE tensor_scalar at 4x, ~0.6us:
                                       Schraudolph bit trick -- the int16
                                       bits ARE bf16(exp(el) * 2^-c); the
                                       constant 2^-c factor cancels in the
                                       softmax normalization)
    msg_h += vt^T @ e2i.bitcast(bf16) (bf16 matmul per head, 1-2 iters
                                       behind; vt row 64 of ones
                                       accumulates the denominator)
  tail (pipelined over two query halves): denominators -> PE ones-
    broadcast -> reciprocal (split DVE approx / ACT table) -> per-head
    multiply -> conv MLP with BN folded into W1/W2 -> residual add.

The bit-trick exp replaces ACT's 2.0us/iter EXP with a 0.6us DVE op at
4x packing, and the PSUM-sourced DVE multiply (1x, 2.7us) becomes a
bf16 SBUF multiply (2x, 1.25us) fed by the ACT copy.  Engines balance
at ~2.1us/iter (PE ~2.1, ACT ~2.1, DVE ~1.9).  keep_warm() dummy
matmuls paper over early PE idle gaps so the HAM activity monitor never
re-throttles the clock mid-kernel.  Numerics: the trick's piecewise-
linear 2^frac has +-3% per-element error which largely cancels between
softmax numerator and denominator (and vt/e2 are now bf16, removing the
old fp8 quantization); end-to-end rel error ~1e-4 vs tolerance 2e-2.
"""
import numpy as np
from contextlib import ExitStack

D = 256
N = 4096
NQ = 512          # queries per core
H = 4
DH = 64
NCORES = 8
NIT = N // 128    # 32 key chunks
VTS = 68          # padded per-head stride in the V_T-aug tile
WS = 16.0         # host prescale on conv weights before fp8 quantization
EC1 = 184.6650    # 2^7 / ln(2): bf16 exponent scale for the exp bit trick
EC2 = 16250.4     # 127*2^7 minus half the interp error (Schraudolph magic)

_CACHE = {}


def _build(has_bq, has_bk, has_bv, has_b3):
    import concourse.bass as bass
    import concourse.tile as tile
    from concourse import bacc, mybir

    F32 = mybir.dt.float32
    BF16 = mybir.dt.bfloat16
    I16 = mybir.dt.int16
    FP8 = mybir.dt.float8e4
    Id = mybir.ActivationFunctionType.Identity
    Relu = mybir.ActivationFunctionType.Relu
    DR = mybir.MatmulPerfMode.DoubleRow
    Mul = mybir.AluOpType.mult
    Add = mybir.AluOpType.add

    nc = bacc.Bacc("TRN2", target_bir_lowering=False, debug=False,
                   num_devices=NCORES)

    # DRAM I/O (per core)
    x8_d = nc.dram_tensor("x8", [4, 128, 2, N // 4], FP8,
                          kind="ExternalInput").ap()
    xq8_d = nc.dram_tensor("xq8", [128, 2, NQ], FP8, kind="ExternalInput").ap()
    xqr_d = nc.dram_tensor("xqr", [D, NQ], F32, kind="ExternalInput").ap()
    spt_d = nc.dram_tensor("spt", [N, NQ], BF16, kind="ExternalInput").ap()
    wq8_d = nc.dram_tensor("wq8", [128, 2, D], FP8, kind="ExternalInput").ap()
    wk8_d = nc.dram_tensor("wk8", [128, 2, D], FP8, kind="ExternalInput").ap()
    wv8_d = nc.dram_tensor("wv8", [128, 2, D], FP8, kind="ExternalInput").ap()
    w1t_d = nc.dram_tensor("w1t", [D, 128], BF16, kind="ExternalInput").ap()
    w2t_d = nc.dram_tensor("w2t", [128, 128], BF16, kind="ExternalInput").ap()
    w3t_d = nc.dram_tensor("w3t", [128, D], BF16, kind="ExternalInput").ap()
    bq_d = nc.dram_tensor("bq2", [128, 2], F32, kind="ExternalInput").ap()
    bk_d = nc.dram_tensor("bk2", [128, 2], F32, kind="ExternalInput").ap()
    bv_d = nc.dram_tensor("bv2", [128, 2], F32, kind="ExternalInput").ap()
    b1_d = nc.dram_tensor("b1f", [128, 1], F32, kind="ExternalInput").ap()
    b2_d = nc.dram_tensor("b2f", [128, 1], F32, kind="ExternalInput").ap()
    b3_d = nc.dram_tensor("b32", [128, 2], F32, kind="ExternalInput").ap()
    out_d = nc.dram_tensor("out", [D, NQ], F32, kind="ExternalOutput").ap()

    spt_t3 = spt_d.rearrange("(t p) o -> t p o", p=128)

    with tile.TileContext(nc) as tc, ExitStack() as ctx:
        sb = ctx.enter_context(tc.tile_pool(name="sb", bufs=1))
        spt_pool = ctx.enter_context(tc.tile_pool(name="sptp", bufs=10))
        sc_pool = ctx.enter_context(tc.tile_pool(name="scp", bufs=3))
        el_pool = ctx.enter_context(tc.tile_pool(name="elp", bufs=3))
        e2_pool = ctx.enter_context(tc.tile_pool(name="e2p", bufs=4))
        pj_ctx = ExitStack()
        pj = pj_ctx.enter_context(tc.tile_pool(name="pj", bufs=4, space="PSUM"))

        # ---- early inputs. One HWDGE queue sustains only ~60 GB/s, so the
        # startup transfers are spread over the three rings: sync takes the
        # x quarters, scalar the small weight tensors plus half of quarter 0,
        # gpsimd the other half before its spt stream. ----
        x8t = [sb.tile([128, 2, 1024], FP8, name=f"x8_{k}") for k in range(4)]
        for k in (0, 1, 2):
            nc.sync.dma_start(x8t[k][:, 0, :], x8_d[k][:, 0, :])
        nc.sync.dma_start(x8t[2][:, 1, :], x8_d[2][:, 1, :])
        nc.sync.dma_start(x8t[3][:, 0, :], x8_d[3][:, 0, :])
        for k in (0, 1, 3):
            nc.gpsimd.dma_start(x8t[k][:, 1, :], x8_d[k][:, 1, :])
        wq8 = sb.tile([128, 2, D], FP8, name="wq8")
        wk8 = sb.tile([128, 2, D], FP8, name="wk8")
        wv8 = sb.tile([128, 2, D], FP8, name="wv8")
        nc.scalar.dma_start(wk8[:], wk8_d[:, :, :])
        nc.scalar.dma_start(wv8[:], wv8_d[:, :, :])
        xq8 = sb.tile([128, 2, NQ], FP8, name="xq8")
        nc.scalar.dma_start(xq8[:], xq8_d[:, :, :])
        nc.scalar.dma_start(wq8[:], wq8_d[:, :, :])
        if has_bq:
            bq = sb.tile([128, 2], F32, name="bq")
            nc.scalar.dma_start(bq[:], bq_d[:, :])
        if has_bk:
            bk = sb.tile([128, 2], F32, name="bk")
            nc.scalar.dma_start(bk[:], bk_d[:, :])

        k_sb = [sb.tile([128, 2, NQ], BF16, name=f"ksb{ib}") for ib in range(8)]
        q_sb = [sb.tile([128, NQ], BF16, name=f"q{co}") for co in range(2)]
        # V^T augmented: per key-chunk it, per head h: [64 V cols | ones | pad]
        vt = [sb.tile([128, 4, H, VTS], BF16, name=f"vt{ib}") for ib in range(8)]
        for ib in range(8):
            nc.gpsimd.memset(vt[ib][:, :, :, 64:65], 1.0)
        ones64 = sb.tile([1, 64], BF16, name="ones64")
        nc.gpsimd.memset(ones64[:], 1.0)

        # ---- PE warmup: ~3.5us of tiny matmuls during the DMA ramp so the
        # HAM clock gate is already at 8/8 when real projections start ----
        warm = sb.tile([128, NQ], BF16, name="warm")
        nc.vector.memset(warm[:].bitcast(F32)[:, 0:256], 0.0)
        wps = pj.tile([128, NQ], F32, tag="t")
        for r in range(12):
            nc.tensor.matmul(wps[0:64, :], warm[:, 0:64], warm[:],
                             start=True, stop=True)

        def keep_warm(ap, n):
            # dummy matmuls into a PSUM region that a later start=True matmul
            # fully overwrites; fills PE idle gaps so the HAM clock stays 8/8
            for r in range(n):
                nc.tensor.matmul(ap, warm[:, 0:64], warm[:, 0:64],
                                 start=True, stop=True)

        # spt prefetch on the (otherwise idle) GPSIMD DMA ring
        spt_tiles = {}

        def load_spt(it):
            # alternate rings: the loop-steady spt stream (2.1us/chunk at
            # ~60GB/s/queue) nearly saturates one ring and jitters the DVE;
            # odd chunks ride the sync ring, which is idle during the loop
            t = spt_pool.tile([128, NQ], BF16, tag="spt")
            if it % 2 == 0:
                nc.gpsimd.dma_start(t[:], spt_t3[it])
            else:
                nc.sync.dma_start(t[:], spt_t3[it])
            spt_tiles[it] = t

        for it in range(6):
            load_spt(it)

        def q_proj():
            # fp8 DoubleRow conv1x1, contraction = 256 channels
            for co in range(2):
                ps = pj.tile([128, NQ], F32, tag="t")
                nc.tensor.matmul(ps[:], wq8[:, :, co * 128:(co + 1) * 128],
                                 xq8[:], start=True, stop=True, perf_mode=DR)
                if has_bq:
                    nc.scalar.activation(q_sb[co][:], ps[:], Id,
                                         scale=1.0 / (WS * 8.0),
                                         bias=bq[:, co:co + 1])
                else:
                    nc.scalar.activation(q_sb[co][:], ps[:], Id,
                                         scale=1.0 / (WS * 8.0))

        # ---- K / V^T projections per 512-key block, copies chase on
        # alternating ACT/DVE ----
        cp = [0]

        def copy_scaled(dst, src, bias=None):
            if bias is not None:
                nc.scalar.activation(dst, src, Id, scale=1.0 / WS, bias=bias)
            elif cp[0] % 2 == 0:
                nc.scalar.activation(dst, src, Id, scale=1.0 / WS)
            else:
                nc.vector.tensor_scalar_mul(dst, src, 1.0 / WS)
            cp[0] += 1

        for ib in range(8):
            xt = x8t[ib // 2]
            xo = (ib % 2) * 512
            kps = pj.tile([128, 2, NQ], F32, tag="t")
            for co in range(2):
                nc.tensor.matmul(kps[:, co, :],
                                 wk8[:, :, co * 128:(co + 1) * 128],
                                 xt[:, :, xo:xo + 512],
                                 start=True, stop=True, perf_mode=DR)
            ksl = k_sb[ib][:]
            if has_bk:
                for co in range(2):
                    nc.scalar.activation(ksl[:, co, :], kps[:, co, :], Id,
                                         scale=1.0 / WS, bias=bk[:, co:co + 1])
            else:
                copy_scaled(ksl, kps[:])
            vps = pj.tile([128, 4, 256], F32, tag="t")
            for u in range(4):
                ko = xo + u * 128
                nc.tensor.matmul(vps[:, u, 0:D], xt[:, :, ko:ko + 128],
                                 wv8[:], start=True, stop=True,
                                 perf_mode=DR)
            vdst = vt[ib][:, 0:4, :, 0:64]
            vsrc = vps[:, 0:4, 0:D].rearrange("p w (h c) -> p w h c", h=H)
            copy_scaled(vdst, vsrc)
            if ib == 1:
                q_proj()
        wps2 = pj.tile([128, NQ], F32, tag="t")
        keep_warm(wps2[0:64, 0:64], 8)

        pj_ctx.close()
        ps_m = ctx.enter_context(tc.tile_pool(name="psm", bufs=1, space="PSUM"))
        pst_ctx = ExitStack()
        ps_t = pst_ctx.enter_context(
            tc.tile_pool(name="pst", bufs=2, space="PSUM"))
        # messages for all 4 heads in one 4-bank PSUM tensor; row 64 of each
        # bank accumulates the softmax denominator (ones column in vt)
        mps = ps_m.tile([65, H, NQ], F32, name="mps")

        def head_bcast(ap, w):
            # broadcast a [128, q] AP over the head axis with a 0-stride dim
            return bass.AP(tensor=ap.tensor, offset=ap.offset,
                           ap=[list(ap.ap[0]), [0, w], list(ap.ap[1])])

        # ---- attention loop ----
        pendq = []

        def emit_msgs(p):
            pit, e2p = p
            e2b = e2p[:].bitcast(BF16)
            for h in range(H):
                nc.tensor.matmul(mps[:, h, :], vt[pit // 4][:, pit % 4, h, 0:65],
                                 e2b[:, h, :],
                                 start=(pit == 0), stop=(pit == NIT - 1),
                                 skip_group_check=True)

        for it in range(NIT):
            if it + 6 < NIT:
                load_spt(it + 6)
            spt_t = spt_tiles.pop(it)
            sc = sc_pool.tile([128, H, NQ], BF16, tag="sc")
            el = el_pool.tile([128, H, NQ], BF16, tag="el")
            e2i = e2_pool.tile([128, H, NQ], I16, tag="e2")
            sps_l = []
            for hp in range(2):
                sps = ps_t.tile([128, 2, NQ], F32, tag="t")
                sps_l.append(sps)
                if it < 2:
                    keep_warm(sps[0:64, 0, 0:64], 8)
                elif it < 4:
                    keep_warm(sps[0:64, 0, 0:64], 3)
                for jj in range(2):
                    ro = jj * 64
                    nc.tensor.matmul(
                        sps[:, jj, :],
                        k_sb[it // 4][ro:ro + 64, hp,
                                      (it % 4) * 128:(it % 4) * 128 + 128],
                        q_sb[hp][ro:ro + 64, :],
                        start=True, stop=True)
                nc.scalar.copy(sc[:, 2 * hp:2 * hp + 2, :], sps[:])
            if it < NIT - 1:
                nc.vector.tensor_mul(el[:], sc[:], head_bcast(spt_t[:], H))
                nc.vector.tensor_scalar(e2i[:], el[:], EC1, EC2, Mul, Add)
                # messages run TWO iterations behind so the producer chain
                # (ACT copy -> DVE TT -> DVE TS) is finished when PE arrives
                if len(pendq) >= 2:
                    emit_msgs(pendq.pop(0))
                pendq.append((it, e2i))
            else:
                # final iteration: drain pending messages, then pipeline the
                # last chunk + tail over two query halves
                for p in pendq:
                    emit_msgs(p)
                pendq = []

        pst_ctx.close()

        # ---- late inputs (only needed after the attention loop) ----
        w1t = [sb.tile([128, 128], BF16, name=f"w1t{ci}") for ci in range(2)]
        for ci in range(2):
            nc.scalar.dma_start(w1t[ci][:], w1t_d[ci * 128:(ci + 1) * 128, :])
        w2t = sb.tile([128, 128], BF16, name="w2t")
        nc.scalar.dma_start(w2t[:], w2t_d[:, :])
        w3t = sb.tile([128, D], BF16, name="w3t")
        nc.scalar.dma_start(w3t[:], w3t_d[:, :])
        xqr = [sb.tile([128, NQ], F32, name=f"xqr{co}") for co in range(2)]
        for co in range(2):
            nc.sync.dma_start(xqr[co][:], xqr_d[co * 128:(co + 1) * 128, :])
        b1 = sb.tile([128, 1], F32, name="b1")
        b2 = sb.tile([128, 1], F32, name="b2")
        nc.scalar.dma_start(b1[:], b1_d[:, :])
        nc.scalar.dma_start(b2[:], b2_d[:, :])
        if has_bv:
            bv = sb.tile([128, 2], F32, name="bv")
            nc.scalar.dma_start(bv[:], bv_d[:, :])
        if has_b3:
            b3 = sb.tile([128, 2], F32, name="b3")
            nc.scalar.dma_start(b3[:], b3_d[:, :])

        # tail tiles
        HQ = NQ // 2
        dhs = sb.tile([1, H, NQ], BF16, name="dhs")
        msg = [sb.tile([128, NQ], BF16, name=f"msg{co}") for co in range(2)]
        rbc = sb.tile([64, H, NQ], F32, name="rbc")
        h1 = sb.tile([128, NQ], BF16, name="h1")
        h2 = sb.tile([128, NQ], BF16, name="h2")
        ot = [sb.tile([128, NQ], F32, name=f"ot{co}") for co in range(2)]
        tb = sb.tile([128, NQ], F32, name="tb")

        pt_ctx = ExitStack()
        pd = pt_ctx.enter_context(tc.tile_pool(name="pd", bufs=1, space="PSUM"))
        pt = pt_ctx.enter_context(tc.tile_pool(name="pt", bufs=2, space="PSUM"))

        def drain_half(qh, sc, el, e2i, spt_t):
            ql = slice(qh * HQ, (qh + 1) * HQ)
            nc.vector.tensor_mul(el[:, :, ql], sc[:, :, ql],
                                 head_bcast(spt_t[:, ql], H))
            nc.vector.tensor_scalar(e2i[:, :, ql], el[:, :, ql], EC1, EC2,
                                    Mul, Add)
            e2b = e2i[:].bitcast(BF16)
            for h in range(H):
                nc.tensor.matmul(mps[:, h, ql], vt[7][:, 3, h, 0:65],
                                 e2b[:, h, ql],
                                 start=False, stop=True,
                                 skip_group_check=True)

        def tail_half(qh):
            # denominators -> PE ones-broadcast -> reciprocal (DVE/ACT split)
            # -> per-head multiply -> conv MLP with BN folded -> residual add
            ql = slice(qh * HQ, (qh + 1) * HQ)
            nc.scalar.copy(dhs[:, 0:2, ql], mps[64:65, 0:2, ql])
            nc.vector.tensor_copy(dhs[:, 2:4, ql], mps[64:65, 2:4, ql])
            dbb = pd.tile([64, H, HQ], F32, tag="d")
            for hp in range(2):
                nc.tensor.matmul(dbb[:, 2 * hp:2 * hp + 2, :], ones64[:],
                                 dhs[:, 2 * hp:2 * hp + 2, ql],
                                 start=True, stop=True)
            nc.vector.reciprocal_approx_fast(out=rbc[:, :, ql], in_=dbb[:])
            for co in range(2):
                for jj in range(2):
                    h = 2 * co + jj
                    ro = jj * 64
                    nc.vector.tensor_mul(msg[co][ro:ro + 64, ql],
                                         mps[0:64, h, ql], rbc[:, h, ql])
                    if has_bv:
                        nc.scalar.activation(msg[co][ro:ro + 64, ql],
                                             msg[co][ro:ro + 64, ql], Id,
                                             bias=bv[ro:ro + 64, co:co + 1])
            u1 = pt.tile([128, HQ], F32, tag="t")
            for ci in range(2):
                nc.tensor.matmul(u1[:], w1t[ci][:], msg[ci][:, ql],
                                 start=(ci == 0), stop=(ci == 1))
            nc.scalar.activation(h1[:, ql], u1[:], Relu, bias=b1[:, 0:1])
            u2 = pt.tile([128, HQ], F32, tag="t")
            nc.tensor.matmul(u2[:], w2t[:], h1[:, ql], start=True, stop=True)
            nc.scalar.activation(h2[:, ql], u2[:], Relu, bias=b2[:, 0:1])
            for co in range(2):
                u3 = pt.tile([128, HQ], F32, tag="t")
                nc.tensor.matmul(u3[:], w3t[:, co * 128:(co + 1) * 128],
                                 h2[:, ql], start=True, stop=True)
                if has_b3:
                    nc.scalar.activation(tb[:, ql], u3[:], Id,
                                         bias=b3[:, co:co + 1])
                    nc.vector.tensor_add(ot[co][:, ql], tb[:, ql],
                                         xqr[co][:, ql])
                else:
                    nc.vector.tensor_add(ot[co][:, ql], u3[:], xqr[co][:, ql])
                # split the 128KB output transfers over two rings so the
                # trailing DMA after the last compute is halved
                ring = nc.sync if co == 0 else nc.gpsimd
                ring.dma_start(out_d[co * 128:(co + 1) * 128, ql],
                               ot[co][:, ql])

        # last-chunk drain + tail, pipelined over the two query halves
        drain_half(0, sc, el, e2i, spt_t)
        drain_half(1, sc, el, e2i, spt_t)
        tail_half(0)
        tail_half(1)
        pt_ctx.close()

    nc.compile()
    return nc


def _prep_inputs(inputs):
    import ml_dtypes
    E4 = ml_dtypes.float8_e4m3
    bf = lambda a: np.ascontiguousarray(
        np.asarray(a, dtype=np.float32).astype(ml_dtypes.bfloat16))
    f8 = lambda a: np.ascontiguousarray(
        np.asarray(a, dtype=np.float32).astype(E4))
    f = lambda a: np.ascontiguousarray(np.asarray(a, dtype=np.float32))
    planar = lambda a: np.ascontiguousarray(
        np.asarray(a, np.float32).reshape(2, 128, -1).transpose(1, 0, 2))

    x32 = f(inputs["corr_feat_belief"][0])                  # [D, N]
    spT = np.asarray(inputs["spatial_compatibility"][0]).T  # [N(keys), N(q)]
    Wq, bq = f(inputs["Wq"]), f(inputs["bq"])
    Wk, bk = f(inputs["Wk"]), f(inputs["bk"])
    Wv, bv = f(inputs["Wv"]), f(inputs["bv"])
    W1, b1, g1, be1 = f(inputs["W1"]), f(inputs["b1"]), f(inputs["g1"]), f(inputs["be1"])
    W2, b2, g2, be2 = f(inputs["W2"]), f(inputs["b2"]), f(inputs["g2"]), f(inputs["be2"])
    W3, b3 = f(inputs["W3"]), f(inputs["b3"])

    scale = np.float32(1.0 / np.sqrt(DH))
    s1 = (g1 / np.sqrt(np.float32(1.0) + np.float32(1e-5))).astype(np.float32)
    s2 = (g2 / np.sqrt(np.float32(1.0) + np.float32(1e-5))).astype(np.float32)

    xpl = planar(x32)               # [128, 2, N]; channel c = p + 128*j
    x8 = f8(xpl)
    # quarter-major so each quarter DMA reads contiguous 2KB/partition lines
    x8q = np.ascontiguousarray(
        np.stack([x8[:, :, k * 1024:(k + 1) * 1024] for k in range(4)]))
    spT_bf = bf(spT)
    common = dict(
        x8=x8q,
        wq8=f8(planar(Wq.T) * WS),
        wk8=f8(planar(Wk.T) * WS),
        wv8=f8(planar(Wv.T) * WS),
        w1t=bf((W1 * s1[:, None]).T),
        w2t=bf((W2 * s2[:, None]).T),
        w3t=bf(W3.T),
        bq2=f((bq * scale).reshape(2, 128).T),
        bk2=f(bk.reshape(2, 128).T),
        bv2=f(bv.reshape(2, 128).T),
        b1f=f((s1 * b1 + be1).reshape(128, 1)),
        b2f=f((s2 * b2 + be2).reshape(128, 1)),
        b32=f(b3.reshape(2, 128).T),
    )
    in_maps = []
    for m in range(NCORES):
        sl = slice(m * NQ, (m + 1) * NQ)
        im = dict(common)
        im["xq8"] = np.ascontiguousarray(x8[:, :, sl])
        im["xqr"] = f(x32[:, sl])
        im["spt"] = np.ascontiguousarray(spT_bf[:, sl])
        in_maps.append(im)
    flags = tuple(bool(np.any(b != 0)) for b in (bq, bk, bv, b3))
    return in_maps, flags


def _run(inputs, trace=False):
    from concourse.bass_utils import run_bass_kernel_spmd
    in_maps, flags = _prep_inputs(inputs)
    if flags not in _CACHE:
        _CACHE[flags] = _build(*flags)
    nc = _CACHE[flags]
    res = run_bass_kernel_spmd(nc, in_maps, core_ids=list(range(NCORES)),
                               trace=trace)
    out = np.concatenate([res.results[m]["out"] for m in range(NCORES)],
                         axis=1)[None]
    return np.ascontiguousarray(out.astype(np.float32)), res


def kernel(**inputs):
    out, _ = _run(inputs, trace=False)
    return out


# revision 20
# speedup vs baseline: 1.1974x; 1.1728x over previous
"""Bass/Tile TRN2 kernel for a non-local attention block (BaseNonLocalBlock).

Contract: kernel(**inputs) takes the FULL inputs of the nn.Module problem
(B=1, D=256, H=4, N=4096) and returns the FULL output [1, 256, 4096].

Sharding: query columns of the N x N attention are split across the 8
NeuronCores (512 queries per core). K/V projections are computed
redundantly on every core (cheap); each core produces its own output
column slice and the host concatenates.

Per-core structure (flash-attention style, scores never hit HBM):
  pre-phase: Q/K/V conv1x1 projections as fp8 DoubleRow matmuls
    (channels packed planar [128, 2, *]; weights prescaled x16 on the
    host, un-scaled for free in the PSUM->SBUF copy scale).  K -> bf16
    per-block tiles; V^T(+ones col per head) -> bf16 per-block tiles.
    Startup DMAs are spread over the sync/scalar/gpsimd rings (one
    HWDGE queue sustains only ~60 GB/s); a ~4us block of dummy matmuls
    warms the PE HAM clock gate (1.2 -> 2.4 GHz) during the DMA ramp.
  loop over 32 key chunks (128 keys each):
    S_T = K_h[:, chunk]^T @ Q_h       (PSUM, 2 heads per row-split pair)
    sc  = copy(S_T)                   (ACT PSUM->SBUF bf16, 2x ~1.0us --
                                       ACT's only loop duty; EXP is gone)
    el  = spt * sc                    (DVE bf16 tensor_tensor at 2x mode,
                                       ~1.25us for all 4 heads)
    e2i = int16(el*184.665 + 16250.4) (DVE tensor_scalar at 4x, ~0.6us:
                                       Schraudolph bit trick -- the int16
                                       bits ARE bf16(exp(el) * 2^-c); the
                                       constant 2^-c factor cancels in the
                                       softmax normalization)
    msg_h += vt^T @ e2i.bitcast(bf16) (bf16 matmul per head, 1-2 iters
                                       behind; vt row 64 of ones
                                       accumulates the denominator)
  tail (pipelined over two query halves): denominators -> PE ones-
    broadcast -> reciprocal (split DVE approx / ACT table) -> per-head
    multiply -> conv MLP with BN folded into W1/W2 -> residual add.

The bit-trick exp replaces ACT's 2.0us/iter EXP with a 0.6us DVE op at
4x packing, and the PSUM-sourced DVE multiply (1x, 2.7us) becomes a
bf16 SBUF multiply (2x, 1.25us) fed by the ACT copy.  Engines balance
at ~2.1us/iter (PE ~2.1, ACT ~2.1, DVE ~1.9).  keep_warm() dummy
matmuls paper over early PE idle gaps so the HAM activity monitor never
re-throttles the clock mid-kernel.  Numerics: the trick's piecewise-
linear 2^frac has +-3% per-element error which largely cancels between
softmax numerator and denominator (and vt/e2 are now bf16, removing the
old fp8 quantization); end-to-end rel error ~1e-4 vs tolerance 2e-2.
"""
import numpy as np
from contextlib import ExitStack

D = 256
N = 4096
NQ = 512          # queries per core
H = 4
DH = 64
NCORES = 8
NIT = N // 128    # 32 key chunks
VTS = 68          # padded per-head stride in the V_T-aug tile
WS = 16.0         # host prescale on conv weights before fp8 quantization
EC1 = 184.6650    # 2^7 / ln(2): bf16 exponent scale for the exp bit trick
EC2 = 16250.4     # 127*2^7 minus half the interp error (Schraudolph magic)

_CACHE = {}


def _build(has_bq, has_bk, has_bv, has_b3):
    import concourse.bass as bass
    import concourse.tile as tile
    from concourse import bacc, mybir

    F32 = mybir.dt.float32
    BF16 = mybir.dt.bfloat16
    I16 = mybir.dt.int16
    FP8 = mybir.dt.float8e4
    Id = mybir.ActivationFunctionType.Identity
    Relu = mybir.ActivationFunctionType.Relu
    DR = mybir.MatmulPerfMode.DoubleRow
    Mul = mybir.AluOpType.mult
    Add = mybir.AluOpType.add

    nc = bacc.Bacc("TRN2", target_bir_lowering=False, debug=False,
                   num_devices=NCORES)

    # DRAM I/O (per core)
    x8_d = nc.dram_tensor("x8", [4, 128, 2, N // 4], FP8,
                          kind="ExternalInput").ap()
    xq8_d = nc.dram_tensor("xq8", [128, 2, NQ], FP8, kind="ExternalInput").ap()
    xqr_d = nc.dram_tensor("xqr", [D, NQ], F32, kind="ExternalInput").ap()
    # spt grouped host-side: [8 groups, 128 partitions, 4 chunks * NQ] so one
    # 512KB dma_start covers 4 key chunks with 4KB contiguous partition lines
    # (the ~2us fixed DMA cost is amortized 4x vs per-chunk transfers)
    spt_d = nc.dram_tensor("spt", [8, 128, 4 * NQ], BF16,
                           kind="ExternalInput").ap()
    wq8_d = nc.dram_tensor("wq8", [128, 2, D], FP8, kind="ExternalInput").ap()
    wk8_d = nc.dram_tensor("wk8", [128, 2, D], FP8, kind="ExternalInput").ap()
    wv8_d = nc.dram_tensor("wv8", [128, 2, D], FP8, kind="ExternalInput").ap()
    w1t_d = nc.dram_tensor("w1t", [D, 128], BF16, kind="ExternalInput").ap()
    w2t_d = nc.dram_tensor("w2t", [128, 128], BF16, kind="ExternalInput").ap()
    w3t_d = nc.dram_tensor("w3t", [128, D], BF16, kind="ExternalInput").ap()
    bq_d = nc.dram_tensor("bq2", [128, 2], F32, kind="ExternalInput").ap()
    bk_d = nc.dram_tensor("bk2", [128, 2], F32, kind="ExternalInput").ap()
    bv_d = nc.dram_tensor("bv2", [128, 2], F32, kind="ExternalInput").ap()
    b1_d = nc.dram_tensor("b1f", [128, 1], F32, kind="ExternalInput").ap()
    b2_d = nc.dram_tensor("b2f", [128, 1], F32, kind="ExternalInput").ap()
    b3_d = nc.dram_tensor("b32", [128, 2], F32, kind="ExternalInput").ap()
    out_d = nc.dram_tensor("out", [D, NQ], F32, kind="ExternalOutput").ap()

    with tile.TileContext(nc) as tc, ExitStack() as ctx:
        sb = ctx.enter_context(tc.tile_pool(name="sb", bufs=1))
        spt_pool = ctx.enter_context(tc.tile_pool(name="sptp", bufs=3))
        sc_pool = ctx.enter_context(tc.tile_pool(name="scp", bufs=3))
        el_pool = ctx.enter_context(tc.tile_pool(name="elp", bufs=3))
        e2_pool = ctx.enter_context(tc.tile_pool(name="e2p", bufs=4))
        pj_ctx = ExitStack()
        pj = pj_ctx.enter_context(tc.tile_pool(name="pj", bufs=4, space="PSUM"))

        # ---- early inputs. One HWDGE queue sustains only ~60 GB/s, so the
        # startup transfers are spread over the three rings: sync takes the
        # x quarters, scalar the small weight tensors plus half of quarter 0,
        # gpsimd the other half before its spt stream. ----
        x8t = [sb.tile([128, 2, 1024], FP8, name=f"x8_{k}") for k in range(4)]
        for k in (0, 2):
            nc.sync.dma_start(x8t[k][:], x8_d[k])
        for k in (1, 3):
            nc.gpsimd.dma_start(x8t[k][:], x8_d[k])
        wq8 = sb.tile([128, 2, D], FP8, name="wq8")
        wk8 = sb.tile([128, 2, D], FP8, name="wk8")
        wv8 = sb.tile([128, 2, D], FP8, name="wv8")
        nc.scalar.dma_start(wk8[:], wk8_d[:, :, :])
        nc.scalar.dma_start(wv8[:], wv8_d[:, :, :])
        xq8 = sb.tile([128, 2, NQ], FP8, name="xq8")
        nc.scalar.dma_start(xq8[:], xq8_d[:, :, :])
        nc.scalar.dma_start(wq8[:], wq8_d[:, :, :])
        if has_bq:
            bq = sb.tile([128, 2], F32, name="bq")
            nc.scalar.dma_start(bq[:], bq_d[:, :])
        if has_bk:
            bk = sb.tile([128, 2], F32, name="bk")
            nc.scalar.dma_start(bk[:], bk_d[:, :])

        k_sb = [sb.tile([128, 2, NQ], BF16, name=f"ksb{ib}") for ib in range(8)]
        q_sb = [sb.tile([128, NQ], BF16, name=f"q{co}") for co in range(2)]
        # V^T augmented: per key-chunk it, per head h: [64 V cols | ones | pad]
        vt = [sb.tile([128, 4, H, VTS], BF16, name=f"vt{ib}") for ib in range(8)]
        for ib in range(8):
            nc.gpsimd.memset(vt[ib][:, :, :, 64:65], 1.0)
        ones64 = sb.tile([1, 64], BF16, name="ones64")
        nc.gpsimd.memset(ones64[:], 1.0)

        # ---- PE warmup: ~3.5us of tiny matmuls during the DMA ramp so the
        # HAM clock gate is already at 8/8 when real projections start ----
        warm = sb.tile([128, NQ], BF16, name="warm")
        nc.vector.memset(warm[:].bitcast(F32)[:, 0:256], 0.0)
        wps = pj.tile([128, NQ], F32, tag="t")
        for r in range(12):
            nc.tensor.matmul(wps[0:64, :], warm[:, 0:64], warm[:],
                             start=True, stop=True)

        def keep_warm(ap, n):
            # dummy matmuls into a PSUM region that a later start=True matmul
            # fully overwrites; fills PE idle gaps so the HAM clock stays 8/8
            for r in range(n):
                nc.tensor.matmul(ap, warm[:, 0:64], warm[:, 0:64],
                                 start=True, stop=True)

        # spt group prefetch, alternating the sync/gpsimd rings
        spt_groups = {}

        def load_spt_group(g):
            t = spt_pool.tile([128, 4, NQ], BF16, tag="spt")
            ring = nc.gpsimd if g % 2 == 0 else nc.sync
            ring.dma_start(t[:].rearrange("p t o -> p (t o)"), spt_d[g])
            spt_groups[g] = t

        for g in range(2):
            load_spt_group(g)

        def q_proj():
            # fp8 DoubleRow conv1x1, contraction = 256 channels
            for co in range(2):
                ps = pj.tile([128, NQ], F32, tag="t")
                nc.tensor.matmul(ps[:], wq8[:, :, co * 128:(co + 1) * 128],
                                 xq8[:], start=True, stop=True, perf_mode=DR)
                if has_bq:
                    nc.scalar.activation(q_sb[co][:], ps[:], Id,
                                         scale=1.0 / (WS * 8.0),
                                         bias=bq[:, co:co + 1])
                else:
                    nc.scalar.activation(q_sb[co][:], ps[:], Id,
                                         scale=1.0 / (WS * 8.0))

        # ---- K / V^T projections per 512-key block, copies chase on
        # alternating ACT/DVE ----
        cp = [0]

        def copy_scaled(dst, src, bias=None):
            if bias is not None:
                nc.scalar.activation(dst, src, Id, scale=1.0 / WS, bias=bias)
            elif cp[0] % 2 == 0:
                nc.scalar.activation(dst, src, Id, scale=1.0 / WS)
            else:
                nc.vector.tensor_scalar_mul(dst, src, 1.0 / WS)
            cp[0] += 1

        for ib in range(8):
            xt = x8t[ib // 2]
            xo = (ib % 2) * 512
            kps = pj.tile([128, 2, NQ], F32, tag="t")
            for co in range(2):
                nc.tensor.matmul(kps[:, co, :],
                                 wk8[:, :, co * 128:(co + 1) * 128],
                                 xt[:, :, xo:xo + 512],
                                 start=True, stop=True, perf_mode=DR)
            ksl = k_sb[ib][:]
            if has_bk:
                for co in range(2):
                    nc.scalar.activation(ksl[:, co, :], kps[:, co, :], Id,
                                         scale=1.0 / WS, bias=bk[:, co:co + 1])
            else:
                copy_scaled(ksl, kps[:])
            vps = pj.tile([128, 4, 256], F32, tag="t")
            for u in range(4):
                ko = xo + u * 128
                nc.tensor.matmul(vps[:, u, 0:D], xt[:, :, ko:ko + 128],
                                 wv8[:], start=True, stop=True,
                                 perf_mode=DR)
            vdst = vt[ib][:, 0:4, :, 0:64]
            vsrc = vps[:, 0:4, 0:D].rearrange("p w (h c) -> p w h c", h=H)
            copy_scaled(vdst, vsrc)
            if ib == 1:
                q_proj()
        wps2 = pj.tile([128, NQ], F32, tag="t")
        keep_warm(wps2[0:64, 0:64], 8)

        pj_ctx.close()
        ps_m = ctx.enter_context(tc.tile_pool(name="psm", bufs=1, space="PSUM"))
        pst_ctx = ExitStack()
        ps_t = pst_ctx.enter_context(
            tc.tile_pool(name="pst", bufs=2, space="PSUM"))
        # messages for all 4 heads in one 4-bank PSUM tensor; row 64 of each
        # bank accumulates the softmax denominator (ones column in vt)
        mps = ps_m.tile([65, H, NQ], F32, name="mps")

        def head_bcast(ap, w):
            # broadcast a [128, q] AP over the head axis with a 0-stride dim
            return bass.AP(tensor=ap.tensor, offset=ap.offset,
                           ap=[list(ap.ap[0]), [0, w], list(ap.ap[1])])

        # ---- attention loop ----
        pendq = []

        def emit_msgs(p):
            pit, e2p = p
            e2b = e2p[:].bitcast(BF16)
            for h in range(H):
                nc.tensor.matmul(mps[:, h, :], vt[pit // 4][:, pit % 4, h, 0:65],
                                 e2b[:, h, :],
                                 start=(pit == 0), stop=(pit == NIT - 1),
                                 skip_group_check=True)

        for it in range(NIT):
            if it % 4 == 0 and it // 4 + 2 < 8:
                load_spt_group(it // 4 + 2)
            spt_t = spt_groups[it // 4][:, it % 4, :]
            sc = sc_pool.tile([128, H, NQ], BF16, tag="sc")
            el = el_pool.tile([128, H, NQ], BF16, tag="el")
            e2i = e2_pool.tile([128, H, NQ], I16, tag="e2")
            sps_l = []
            for hp in range(2):
                sps = ps_t.tile([128, 2, NQ], F32, tag="t")
                sps_l.append(sps)
                if it < 2:
                    keep_warm(sps[0:64, 0, 0:64], 8)
                elif it < 4:
                    keep_warm(sps[0:64, 0, 0:64], 3)
                for jj in range(2):
                    ro = jj * 64
                    nc.tensor.matmul(
                        sps[:, jj, :],
                        k_sb[it // 4][ro:ro + 64, hp,
                                      (it % 4) * 128:(it % 4) * 128 + 128],
                        q_sb[hp][ro:ro + 64, :],
                        start=True, stop=True)
                nc.scalar.copy(sc[:, 2 * hp:2 * hp + 2, :], sps[:])
            if it < NIT - 1:
                nc.vector.tensor_mul(el[:], sc[:], head_bcast(spt_t, H))
                nc.vector.tensor_scalar(e2i[:], el[:], EC1, EC2, Mul, Add)
                # messages run TWO iterations behind so the producer chain
                # (ACT copy -> DVE TT -> DVE TS) is finished when PE arrives
                if len(pendq) >= 2:
                    emit_msgs(pendq.pop(0))
                pendq.append((it, e2i))
            else:
                # final iteration: drain pending messages, then pipeline the
                # last chunk + tail over two query halves
                for p in pendq:
                    emit_msgs(p)
                pendq = []

        pst_ctx.close()

        # ---- late inputs (only needed after the attention loop) ----
        # per-head row slices of W1^T at base partition 0 (PE requires lhsT
        # and rhs to share the base partition)
        w1t4 = sb.tile([64, H, 128], BF16, name="w1t4")
        nc.scalar.dma_start(w1t4[:], w1t_d.rearrange("(h p) o -> p h o", p=64))
        w2t = sb.tile([128, 128], BF16, name="w2t")
        nc.scalar.dma_start(w2t[:], w2t_d[:, :])
        w3t = sb.tile([128, D], BF16, name="w3t")
        nc.scalar.dma_start(w3t[:], w3t_d[:, :])
        xqr = [sb.tile([128, NQ], F32, name=f"xqr{co}") for co in range(2)]
        for co in range(2):
            nc.sync.dma_start(xqr[co][:], xqr_d[co * 128:(co + 1) * 128, :])
        b1 = sb.tile([128, 1], F32, name="b1")
        b2 = sb.tile([128, 1], F32, name="b2")
        nc.scalar.dma_start(b1[:], b1_d[:, :])
        nc.scalar.dma_start(b2[:], b2_d[:, :])
        if has_bv:
            bv = sb.tile([128, 2], F32, name="bv")
            nc.scalar.dma_start(bv[:], bv_d[:, :])
        if has_b3:
            b3 = sb.tile([128, 2], F32, name="b3")
            nc.scalar.dma_start(b3[:], b3_d[:, :])

        # tail tiles
        HQ = NQ // 2
        dhs = sb.tile([1, H, NQ], BF16, name="dhs")
        msg4 = sb.tile([64, H, NQ], BF16, name="msg4")
        rbc = sb.tile([64, H, NQ], F32, name="rbc")
        h1 = sb.tile([128, NQ], BF16, name="h1")
        h2 = sb.tile([128, NQ], BF16, name="h2")
        ot = [sb.tile([128, NQ], F32, name=f"ot{co}") for co in range(2)]
        tb = sb.tile([128, NQ], F32, name="tb")

        pt_ctx = ExitStack()
        pd = pt_ctx.enter_context(tc.tile_pool(name="pd", bufs=1, space="PSUM"))
        pt = pt_ctx.enter_context(tc.tile_pool(name="pt", bufs=2, space="PSUM"))

        def drain_half(qh, sc, el, e2i, spt_t):
            ql = slice(qh * HQ, (qh + 1) * HQ)
            nc.vector.tensor_mul(el[:, :, ql], sc[:, :, ql],
                                 head_bcast(spt_t[:, ql], H))
            nc.vector.tensor_scalar(e2i[:, :, ql], el[:, :, ql], EC1, EC2,
                                    Mul, Add)
            e2b = e2i[:].bitcast(BF16)
            for h in range(H):
                nc.tensor.matmul(mps[:, h, ql], vt[7][:, 3, h, 0:65],
                                 e2b[:, h, ql],
                                 start=False, stop=True,
                                 skip_group_check=True)

        def tail_half(qh):
            # denominators -> PE ones-broadcast -> reciprocal (DVE/ACT split)
            # -> per-head multiply -> conv MLP with BN folded -> residual add
            ql = slice(qh * HQ, (qh + 1) * HQ)
            nc.scalar.copy(dhs[:, 0:2, ql], mps[64:65, 0:2, ql])
            nc.vector.tensor_copy(dhs[:, 2:4, ql], mps[64:65, 2:4, ql])
            dbb = pd.tile([64, H, HQ], F32, tag="d")
            for hp in range(2):
                nc.tensor.matmul(dbb[:, 2 * hp:2 * hp + 2, :], ones64[:],
                                 dhs[:, 2 * hp:2 * hp + 2, ql],
                                 start=True, stop=True)
            nc.vector.reciprocal_approx_fast(out=rbc[:, :, ql], in_=dbb[:])
            # normalize 2 heads per DVE op; the W1 matmul contracts each
            # head's 64 channels from its own row slice of w1t
            for co in range(2):
                nc.vector.tensor_mul(msg4[:, 2 * co:2 * co + 2, ql],
                                     mps[0:64, 2 * co:2 * co + 2, ql],
                                     rbc[:, 2 * co:2 * co + 2, ql])
            if has_bv:
                for h in range(H):
                    ro = (h % 2) * 64
                    nc.scalar.activation(msg4[:, h, ql], msg4[:, h, ql], Id,
                                         bias=bv[ro:ro + 64, h // 2:h // 2 + 1])
            u1 = pt.tile([128, HQ], F32, tag="t")
            for h in range(H):
                nc.tensor.matmul(u1[:], w1t4[:, h, :], msg4[:, h, ql],
                                 start=(h == 0), stop=(h == H - 1))
            nc.scalar.activation(h1[:, ql], u1[:], Relu, bias=b1[:, 0:1])
            u2 = pt.tile([128, HQ], F32, tag="t")
            nc.tensor.matmul(u2[:], w2t[:], h1[:, ql], start=True, stop=True)
            nc.scalar.activation(h2[:, ql], u2[:], Relu, bias=b2[:, 0:1])
            for co in range(2):
                u3 = pt.tile([128, HQ], F32, tag="t")
                nc.tensor.matmul(u3[:], w3t[:, co * 128:(co + 1) * 128],
                                 h2[:, ql], start=True, stop=True)
                if has_b3:
                    nc.scalar.activation(tb[:, ql], u3[:], Id,
                                         bias=b3[:, co:co + 1])
                    nc.vector.tensor_add(ot[co][:, ql], tb[:, ql],
                                         xqr[co][:, ql])
                else:
                    nc.vector.tensor_add(ot[co][:, ql], u3[:], xqr[co][:, ql])
                # split the 128KB output transfers over two rings so the
                # trailing DMA after the last compute is halved
                ring = nc.sync if co == 0 else nc.gpsimd
                ring.dma_start(out_d[co * 128:(co + 1) * 128, ql],
                               ot[co][:, ql])

        # last-chunk drain + tail, pipelined over the two query halves
        drain_half(0, sc, el, e2i, spt_t)
        drain_half(1, sc, el, e2i, spt_t)
        tail_half(0)
        tail_half(1)
        pt_ctx.close()

    nc.compile()
    return nc


def _prep_inputs(inputs):
    import ml_dtypes
    E4 = ml_dtypes.float8_e4m3
    bf = lambda a: np.ascontiguousarray(
        np.asarray(a, dtype=np.float32).astype(ml_dtypes.bfloat16))
    f8 = lambda a: np.ascontiguousarray(
        np.asarray(a, dtype=np.float32).astype(E4))
    f = lambda a: np.ascontiguousarray(np.asarray(a, dtype=np.float32))
    planar = lambda a: np.ascontiguousarray(
        np.asarray(a, np.float32).reshape(2, 128, -1).transpose(1, 0, 2))

    x32 = f(inputs["corr_feat_belief"][0])                  # [D, N]
    spT = np.asarray(inputs["spatial_compatibility"][0]).T  # [N(keys), N(q)]
    Wq, bq = f(inputs["Wq"]), f(inputs["bq"])
    Wk, bk = f(inputs["Wk"]), f(inputs["bk"])
    Wv, bv = f(inputs["Wv"]), f(inputs["bv"])
    W1, b1, g1, be1 = f(inputs["W1"]), f(inputs["b1"]), f(inputs["g1"]), f(inputs["be1"])
    W2, b2, g2, be2 = f(inputs["W2"]), f(inputs["b2"]), f(inputs["g2"]), f(inputs["be2"])
    W3, b3 = f(inputs["W3"]), f(inputs["b3"])

    scale = np.float32(1.0 / np.sqrt(DH))
    s1 = (g1 / np.sqrt(np.float32(1.0) + np.float32(1e-5))).astype(np.float32)
    s2 = (g2 / np.sqrt(np.float32(1.0) + np.float32(1e-5))).astype(np.float32)

    xpl = planar(x32)               # [128, 2, N]; channel c = p + 128*j
    x8 = f8(xpl)
    # quarter-major so each quarter DMA reads contiguous 2KB/partition lines
    x8q = np.ascontiguousarray(
        np.stack([x8[:, :, k * 1024:(k + 1) * 1024] for k in range(4)]))
    spT_bf = bf(spT)
    common = dict(
        x8=x8q,
        wq8=f8(planar(Wq.T) * WS),
        wk8=f8(planar(Wk.T) * WS),
        wv8=f8(planar(Wv.T) * WS),
        w1t=bf((W1 * s1[:, None]).T),
        w2t=bf((W2 * s2[:, None]).T),
        w3t=bf(W3.T),
        bq2=f((bq * scale).reshape(2, 128).T),
        bk2=f(bk.reshape(2, 128).T),
        bv2=f(bv.reshape(2, 128).T),
        b1f=f((s1 * b1 + be1).reshape(128, 1)),
        b2f=f((s2 * b2 + be2).reshape(128, 1)),
        b32=f(b3.reshape(2, 128).T),
    )
    in_maps = []
    for m in range(NCORES):
        sl = slice(m * NQ, (m + 1) * NQ)
        im = dict(common)
        im["xq8"] = np.ascontiguousarray(x8[:, :, sl])
        im["xqr"] = f(x32[:, sl])
        # group 4 key chunks per DMA: [8, 128, 4*NQ] with 4KB partition lines
        im["spt"] = np.ascontiguousarray(
            spT_bf[:, sl].reshape(8, 4, 128, NQ).transpose(0, 2, 1, 3)
            .reshape(8, 128, 4 * NQ))
        in_maps.append(im)
    flags = tuple(bool(np.any(b != 0)) for b in (bq, bk, bv, b3))
    return in_maps, flags


def _run(inputs, trace=False):
    from concourse.bass_utils import run_bass_kernel_spmd
    in_maps, flags = _prep_inputs(inputs)
    if flags not in _CACHE:
        _CACHE[flags] = _build(*flags)
    nc = _CACHE[flags]
    res = run_bass_kernel_spmd(nc, in_maps, core_ids=list(range(NCORES)),
                               trace=trace)
    out = np.concatenate([res.results[m]["out"] for m in range(NCORES)],
                         axis=1)[None]
    return np.ascontiguousarray(out.astype(np.float32)), res


def kernel(**inputs):
    out, _ = _run(inputs, trace=False)
    return out


# revision 27
# speedup vs baseline: 1.2060x; 1.0071x over previous
"""Bass/Tile TRN2 kernel for a non-local attention block (BaseNonLocalBlock).

Contract: kernel(**inputs) takes the FULL inputs of the nn.Module problem
(B=1, D=256, H=4, N=4096) and returns the FULL output [1, 256, 4096].

Sharding: query columns of the N x N attention are split across the 8
NeuronCores (512 queries per core). K/V projections are computed
redundantly on every core (cheap); each core produces its own output
column slice and the host concatenates.

Per-core structure (flash-attention style, scores never hit HBM):
  pre-phase: Q/K/V conv1x1 projections as fp8 DoubleRow matmuls
    (channels packed planar [128, 2, *]; weights prescaled x16 on the
    host, un-scaled for free in the PSUM->SBUF copy scale).  K -> bf16
    per-block tiles; V^T(+ones col per head) -> bf16 per-block tiles.
    Startup DMAs are spread over the sync/scalar/gpsimd rings (one
    HWDGE queue sustains only ~60 GB/s); a ~4us block of dummy matmuls
    warms the PE HAM clock gate (1.2 -> 2.4 GHz) during the DMA ramp.
  loop over 32 key chunks (128 keys each):
    S_T = K_h[:, chunk]^T @ Q_h       (PSUM, 2 heads per row-split pair)
    sc  = copy(S_T)                   (ACT PSUM->SBUF bf16, 2x ~1.0us --
                                       ACT's only loop duty; EXP is gone)
    el  = spt * sc                    (DVE bf16 tensor_tensor at 2x mode,
                                       ~1.25us for all 4 heads)
    e2i = int16(el*184.665 + 16250.4) (DVE tensor_scalar at 4x, ~0.6us:
                                       Schraudolph bit trick -- the int16
                                       bits ARE bf16(exp(el) * 2^-c); the
                                       constant 2^-c factor cancels in the
                                       softmax normalization)
    msg_h += vt^T @ e2i.bitcast(bf16) (bf16 matmul per head, 1-2 iters
                                       behind; vt row 64 of ones
                                       accumulates the denominator)
  tail (pipelined over two query halves): denominators -> PE ones-
    broadcast -> reciprocal (split DVE approx / ACT table) -> per-head
    multiply -> conv MLP with BN folded into W1/W2 -> residual add.

The bit-trick exp replaces ACT's 2.0us/iter EXP with a 0.6us DVE op at
4x packing, and the PSUM-sourced DVE multiply (1x, 2.7us) becomes a
bf16 SBUF multiply (2x, 1.25us) fed by the ACT copy.  Engines balance
at ~2.1us/iter (PE ~2.1, ACT ~2.1, DVE ~1.9).  keep_warm() dummy
matmuls paper over early PE idle gaps so the HAM activity monitor never
re-throttles the clock mid-kernel.  Numerics: the trick's piecewise-
linear 2^frac has +-3% per-element error which largely cancels between
softmax numerator and denominator (and vt/e2 are now bf16, removing the
old fp8 quantization); end-to-end rel error ~1e-4 vs tolerance 2e-2.
"""
import numpy as np
from contextlib import ExitStack

D = 256
N = 4096
NQ = 512          # queries per core
H = 4
DH = 64
NCORES = 8
NIT = N // 128    # 32 key chunks
VTS = 68          # padded per-head stride in the V_T-aug tile
WS = 16.0         # host prescale on conv weights before fp8 quantization
EC1 = 184.6650    # 2^7 / ln(2): bf16 exponent scale for the exp bit trick
EC2 = 16250.4     # 127*2^7 minus half the interp error (Schraudolph magic)

_CACHE = {}


def _build(has_bq, has_bk, has_bv, has_b3):
    import concourse.bass as bass
    import concourse.tile as tile
    from concourse import bacc, mybir

    F32 = mybir.dt.float32
    BF16 = mybir.dt.bfloat16
    I16 = mybir.dt.int16
    FP8 = mybir.dt.float8e4
    Id = mybir.ActivationFunctionType.Identity
    Relu = mybir.ActivationFunctionType.Relu
    DR = mybir.MatmulPerfMode.DoubleRow
    Mul = mybir.AluOpType.mult
    Add = mybir.AluOpType.add

    nc = bacc.Bacc("TRN2", target_bir_lowering=False, debug=False,
                   num_devices=NCORES)

    # DRAM I/O (per core)
    x8_d = nc.dram_tensor("x8", [4, 128, 2, N // 4], FP8,
                          kind="ExternalInput").ap()
    xq8_d = nc.dram_tensor("xq8", [128, 2, NQ], FP8, kind="ExternalInput").ap()
    xqr_d = nc.dram_tensor("xqr", [D, NQ], F32, kind="ExternalInput").ap()
    # spt grouped host-side: [8 groups, 128 partitions, 4 chunks * NQ] so one
    # 512KB dma_start covers 4 key chunks with 4KB contiguous partition lines
    # (the ~2us fixed DMA cost is amortized 4x vs per-chunk transfers)
    spt_d = nc.dram_tensor("spt", [8, 128, 4 * NQ], BF16,
                           kind="ExternalInput").ap()
    wq8_d = nc.dram_tensor("wq8", [128, 2, D], FP8, kind="ExternalInput").ap()
    wk8_d = nc.dram_tensor("wk8", [128, 2, D], FP8, kind="ExternalInput").ap()
    wv8_d = nc.dram_tensor("wv8", [128, 2, D], FP8, kind="ExternalInput").ap()
    w1t_d = nc.dram_tensor("w1t", [D, 128], BF16, kind="ExternalInput").ap()
    w2t_d = nc.dram_tensor("w2t", [128, 128], BF16, kind="ExternalInput").ap()
    w3t_d = nc.dram_tensor("w3t", [128, D], BF16, kind="ExternalInput").ap()
    bq_d = nc.dram_tensor("bq2", [128, 2], F32, kind="ExternalInput").ap()
    bk_d = nc.dram_tensor("bk2", [128, 2], F32, kind="ExternalInput").ap()
    bv_d = nc.dram_tensor("bv2", [128, 2], F32, kind="ExternalInput").ap()
    b1_d = nc.dram_tensor("b1f", [128, 1], F32, kind="ExternalInput").ap()
    b2_d = nc.dram_tensor("b2f", [128, 1], F32, kind="ExternalInput").ap()
    b3_d = nc.dram_tensor("b32", [128, 2], F32, kind="ExternalInput").ap()
    out_d = nc.dram_tensor("out", [D, NQ], F32, kind="ExternalOutput").ap()

    with tile.TileContext(nc) as tc, ExitStack() as ctx:
        sb = ctx.enter_context(tc.tile_pool(name="sb", bufs=1))
        spt_pool = ctx.enter_context(tc.tile_pool(name="sptp", bufs=3))
        sc_pool = ctx.enter_context(tc.tile_pool(name="scp", bufs=3))
        el_pool = ctx.enter_context(tc.tile_pool(name="elp", bufs=3))
        e2_pool = ctx.enter_context(tc.tile_pool(name="e2p", bufs=4))
        pj_ctx = ExitStack()
        pj = pj_ctx.enter_context(tc.tile_pool(name="pj", bufs=4, space="PSUM"))

        # ---- early inputs. One HWDGE queue sustains only ~60 GB/s, so the
        # startup transfers are spread over the three rings: sync takes the
        # x quarters, scalar the small weight tensors plus half of quarter 0,
        # gpsimd the other half before its spt stream. ----
        # all bulk DMA rides the two HWDGE rings; the gpsimd SWDGE ring is
        # left unused (its descriptor-gen and sem teardown cost more)
        x8t = [sb.tile([128, 2, 1024], FP8, name=f"x8_{k}") for k in range(4)]
        for k in (0, 1):
            nc.sync.dma_start(x8t[k][:], x8_d[k])
        for k in (2, 3):
            nc.scalar.dma_start(x8t[k][:], x8_d[k])
        wq8 = sb.tile([128, 2, D], FP8, name="wq8")
        wk8 = sb.tile([128, 2, D], FP8, name="wk8")
        wv8 = sb.tile([128, 2, D], FP8, name="wv8")
        nc.scalar.dma_start(wk8[:], wk8_d[:, :, :])
        nc.scalar.dma_start(wv8[:], wv8_d[:, :, :])
        xq8 = sb.tile([128, 2, NQ], FP8, name="xq8")
        nc.scalar.dma_start(xq8[:], xq8_d[:, :, :])
        nc.scalar.dma_start(wq8[:], wq8_d[:, :, :])
        if has_bq:
            bq = sb.tile([128, 2], F32, name="bq")
            nc.scalar.dma_start(bq[:], bq_d[:, :])
        if has_bk:
            bk = sb.tile([128, 2], F32, name="bk")
            nc.scalar.dma_start(bk[:], bk_d[:, :])

        k_sb = [sb.tile([128, 2, NQ], BF16, name=f"ksb{ib}") for ib in range(8)]
        q_sb = [sb.tile([128, NQ], BF16, name=f"q{co}") for co in range(2)]
        # V^T augmented: per key-chunk it, per head h: [64 V cols | ones | pad]
        vt = [sb.tile([128, 4, H, VTS], BF16, name=f"vt{ib}") for ib in range(8)]
        for ib in range(8):
            nc.gpsimd.memset(vt[ib][:, :, :, 64:65], 1.0)
        ones64 = sb.tile([1, 64], BF16, name="ones64")
        nc.gpsimd.memset(ones64[:], 1.0)

        # ---- PE warmup: ~3.5us of tiny matmuls during the DMA ramp so the
        # HAM clock gate is already at 8/8 when real projections start ----
        warm = sb.tile([128, NQ], BF16, name="warm")
        nc.vector.memset(warm[:].bitcast(F32)[:, 0:256], 0.0)
        wps = pj.tile([128, NQ], F32, tag="t")
        for r in range(6):
            nc.tensor.matmul(wps[0:64, :], warm[:, 0:64], warm[:],
                             start=True, stop=True)

        def keep_warm(ap, n):
            # dummy matmuls into a PSUM region that a later start=True matmul
            # fully overwrites; fills PE idle gaps so the HAM clock stays 8/8
            for r in range(n):
                nc.tensor.matmul(ap, warm[:, 0:64], warm[:, 0:64],
                                 start=True, stop=True)

        # spt group prefetch: all on the sync ring, which is otherwise idle
        # during the loop (one 512KB group every ~3.2us vs ~8.2us consumption)
        spt_groups = {}

        def load_spt_group(g):
            t = spt_pool.tile([128, 4, NQ], BF16, tag="spt")
            nc.sync.dma_start(t[:].rearrange("p t o -> p (t o)"), spt_d[g])
            spt_groups[g] = t

        for g in range(2):
            load_spt_group(g)

        def q_proj():
            # fp8 DoubleRow conv1x1, contraction = 256 channels
            for co in range(2):
                ps = pj.tile([128, NQ], F32, tag="t")
                nc.tensor.matmul(ps[:], wq8[:, :, co * 128:(co + 1) * 128],
                                 xq8[:], start=True, stop=True, perf_mode=DR)
                if has_bq:
                    nc.scalar.activation(q_sb[co][:], ps[:], Id,
                                         scale=1.0 / (WS * 8.0),
                                         bias=bq[:, co:co + 1])
                else:
                    nc.scalar.activation(q_sb[co][:], ps[:], Id,
                                         scale=1.0 / (WS * 8.0))

        # ---- K / V^T projections per 512-key block, copies chase on
        # alternating ACT/DVE ----
        cp = [0]

        def copy_scaled(dst, src, bias=None):
            if bias is not None:
                nc.scalar.activation(dst, src, Id, scale=1.0 / WS, bias=bias)
            elif cp[0] % 2 == 0:
                nc.scalar.activation(dst, src, Id, scale=1.0 / WS)
            else:
                nc.vector.tensor_scalar_mul(dst, src, 1.0 / WS)
            cp[0] += 1

        for ib in range(8):
            xt = x8t[ib // 2]
            xo = (ib % 2) * 512
            kps = pj.tile([128, 2, NQ], F32, tag="t")
            for co in range(2):
                nc.tensor.matmul(kps[:, co, :],
                                 wk8[:, :, co * 128:(co + 1) * 128],
                                 xt[:, :, xo:xo + 512],
                                 start=True, stop=True, perf_mode=DR)
            ksl = k_sb[ib][:]
            if has_bk:
                for co in range(2):
                    nc.scalar.activation(ksl[:, co, :], kps[:, co, :], Id,
                                         scale=1.0 / WS, bias=bk[:, co:co + 1])
            else:
                copy_scaled(ksl, kps[:])
            vps = pj.tile([128, 4, 256], F32, tag="t")
            for u in range(4):
                ko = xo + u * 128
                nc.tensor.matmul(vps[:, u, 0:D], xt[:, :, ko:ko + 128],
                                 wv8[:], start=True, stop=True,
                                 perf_mode=DR)
            vdst = vt[ib][:, 0:4, :, 0:64]
            vsrc = vps[:, 0:4, 0:D].rearrange("p w (h c) -> p w h c", h=H)
            copy_scaled(vdst, vsrc)
        # q is only needed by the loop itself; projecting it last keeps the
        # pre-phase PSUM slot rotation from serializing on the q tiles
        q_proj()
        wps2 = pj.tile([128, NQ], F32, tag="t")
        keep_warm(wps2[0:64, 0:64], 4)

        pj_ctx.close()
        ps_m = ctx.enter_context(tc.tile_pool(name="psm", bufs=1, space="PSUM"))
        pst_ctx = ExitStack()
        ps_t = pst_ctx.enter_context(
            tc.tile_pool(name="pst", bufs=2, space="PSUM"))
        # messages for all 4 heads in one 4-bank PSUM tensor; row 64 of each
        # bank accumulates the softmax denominator (ones column in vt)
        mps = ps_m.tile([65, H, NQ], F32, name="mps")

        def head_bcast(ap, w):
            # broadcast a [128, q] AP over the head axis with a 0-stride dim
            return bass.AP(tensor=ap.tensor, offset=ap.offset,
                           ap=[list(ap.ap[0]), [0, w], list(ap.ap[1])])

        # ---- attention loop ----
        pendq = []

        def emit_msgs(p):
            pit, e2p = p
            e2b = e2p[:].bitcast(BF16)
            for h in range(H):
                nc.tensor.matmul(mps[:, h, :], vt[pit // 4][:, pit % 4, h, 0:65],
                                 e2b[:, h, :],
                                 start=(pit == 0), stop=(pit == NIT - 1),
                                 skip_group_check=True)

        for it in range(NIT):
            if it % 4 == 0 and it // 4 + 2 < 8:
                load_spt_group(it // 4 + 2)
            spt_t = spt_groups[it // 4][:, it % 4, :]
            sc = sc_pool.tile([128, H, NQ], BF16, tag="sc")
            el = el_pool.tile([128, H, NQ], BF16, tag="el")
            e2i = e2_pool.tile([128, H, NQ], I16, tag="e2")
            sps_l = []
            for hp in range(2):
                sps = ps_t.tile([128, 2, NQ], F32, tag="t")
                sps_l.append(sps)
                if it < 2:
                    keep_warm(sps[0:64, 0, 0:64], 8)
                elif it < 4:
                    keep_warm(sps[0:64, 0, 0:64], 3)
                for jj in range(2):
                    ro = jj * 64
                    nc.tensor.matmul(
                        sps[:, jj, :],
                        k_sb[it // 4][ro:ro + 64, hp,
                                      (it % 4) * 128:(it % 4) * 128 + 128],
                        q_sb[hp][ro:ro + 64, :],
                        start=True, stop=True)
                nc.scalar.copy(sc[:, 2 * hp:2 * hp + 2, :], sps[:])
            if it < NIT - 1:
                nc.vector.tensor_mul(el[:], sc[:], head_bcast(spt_t, H))
                nc.vector.tensor_scalar(e2i[:], el[:], EC1, EC2, Mul, Add)
                # messages run TWO iterations behind so the producer chain
                # (ACT copy -> DVE TT -> DVE TS) is finished when PE arrives
                if len(pendq) >= 2:
                    emit_msgs(pendq.pop(0))
                pendq.append((it, e2i))
            else:
                # final iteration: drain pending messages, then pipeline the
                # last chunk + tail over two query halves
                for p in pendq:
                    emit_msgs(p)
                pendq = []

        pst_ctx.close()

        # ---- late inputs (only needed after the attention loop) ----
        # per-head row slices of W1^T at base partition 0 (PE requires lhsT
        # and rhs to share the base partition).  All late inputs ride the
        # sync ring: its sequencer is idle by now, while a DIRECT2D on the
        # scalar ring would stall the ACT datapath right at tail start.
        w1t4 = sb.tile([64, H, 128], BF16, name="w1t4")
        nc.sync.dma_start(w1t4[:], w1t_d.rearrange("(h p) o -> p h o", p=64))
        w2t = sb.tile([128, 128], BF16, name="w2t")
        nc.sync.dma_start(w2t[:], w2t_d[:, :])
        w3t = sb.tile([128, D], BF16, name="w3t")
        nc.sync.dma_start(w3t[:], w3t_d[:, :])
        xqr = [sb.tile([128, NQ], F32, name=f"xqr{co}") for co in range(2)]
        for co in range(2):
            nc.sync.dma_start(xqr[co][:], xqr_d[co * 128:(co + 1) * 128, :])
        b1 = sb.tile([128, 1], F32, name="b1")
        b2 = sb.tile([128, 1], F32, name="b2")
        nc.sync.dma_start(b1[:], b1_d[:, :])
        nc.sync.dma_start(b2[:], b2_d[:, :])
        if has_bv:
            bv = sb.tile([128, 2], F32, name="bv")
            nc.sync.dma_start(bv[:], bv_d[:, :])
        if has_b3:
            b3 = sb.tile([128, 2], F32, name="b3")
            nc.sync.dma_start(b3[:], b3_d[:, :])

        # tail tiles
        HQ = NQ // 2
        dhs = sb.tile([1, H, NQ], BF16, name="dhs")
        msg4 = sb.tile([64, H, NQ], BF16, name="msg4")
        rbc = sb.tile([64, H, NQ], F32, name="rbc")
        h1 = sb.tile([128, NQ], BF16, name="h1")
        h2 = sb.tile([128, NQ], BF16, name="h2")
        ot = [sb.tile([128, NQ], F32, name=f"ot{co}") for co in range(2)]
        tb = sb.tile([128, NQ], F32, name="tb")

        pt_ctx = ExitStack()
        pd = pt_ctx.enter_context(tc.tile_pool(name="pd", bufs=1, space="PSUM"))
        pt = pt_ctx.enter_context(tc.tile_pool(name="pt", bufs=2, space="PSUM"))

        def drain_half(qh, sc, el, e2i, spt_t):
            ql = slice(qh * HQ, (qh + 1) * HQ)
            nc.vector.tensor_mul(el[:, :, ql], sc[:, :, ql],
                                 head_bcast(spt_t[:, ql], H))
            nc.vector.tensor_scalar(e2i[:, :, ql], el[:, :, ql], EC1, EC2,
                                    Mul, Add)
            e2b = e2i[:].bitcast(BF16)
            for h in range(H):
                nc.tensor.matmul(mps[:, h, ql], vt[7][:, 3, h, 0:65],
                                 e2b[:, h, ql],
                                 start=False, stop=True,
                                 skip_group_check=True)

        def tail_half(qh):
            # denominators -> PE ones-broadcast -> reciprocal (DVE/ACT split)
            # -> per-head multiply -> conv MLP with BN folded -> residual add
            ql = slice(qh * HQ, (qh + 1) * HQ)
            # both denominator copies on ACT: the DVE is the tail bottleneck
            nc.scalar.copy(dhs[:, 0:2, ql], mps[64:65, 0:2, ql])
            nc.scalar.copy(dhs[:, 2:4, ql], mps[64:65, 2:4, ql])
            dbb = pd.tile([64, H, HQ], F32, tag="d")
            for hp in range(2):
                nc.tensor.matmul(dbb[:, 2 * hp:2 * hp + 2, :], ones64[:],
                                 dhs[:, 2 * hp:2 * hp + 2, ql],
                                 start=True, stop=True)
            nc.vector.reciprocal_approx_fast(out=rbc[:, :, ql], in_=dbb[:])
            # normalize 2 heads per DVE op; the W1 matmul contracts each
            # head's 64 channels from its own row slice of w1t
            for co in range(2):
                nc.vector.tensor_mul(msg4[:, 2 * co:2 * co + 2, ql],
                                     mps[0:64, 2 * co:2 * co + 2, ql],
                                     rbc[:, 2 * co:2 * co + 2, ql])
            if has_bv:
                for h in range(H):
                    ro = (h % 2) * 64
                    nc.scalar.activation(msg4[:, h, ql], msg4[:, h, ql], Id,
                                         bias=bv[ro:ro + 64, h // 2:h // 2 + 1])
            u1 = pt.tile([128, HQ], F32, tag="t")
            for h in range(H):
                nc.tensor.matmul(u1[:], w1t4[:, h, :], msg4[:, h, ql],
                                 start=(h == 0), stop=(h == H - 1))
            nc.scalar.activation(h1[:, ql], u1[:], Relu, bias=b1[:, 0:1])
            u2 = pt.tile([128, HQ], F32, tag="t")
            nc.tensor.matmul(u2[:], w2t[:], h1[:, ql], start=True, stop=True)
            nc.scalar.activation(h2[:, ql], u2[:], Relu, bias=b2[:, 0:1])
            for co in range(2):
                u3 = pt.tile([128, HQ], F32, tag="t")
                nc.tensor.matmul(u3[:], w3t[:, co * 128:(co + 1) * 128],
                                 h2[:, ql], start=True, stop=True)
                if has_b3:
                    nc.scalar.activation(tb[:, ql], u3[:], Id,
                                         bias=b3[:, co:co + 1])
                    nc.vector.tensor_add(ot[co][:, ql], tb[:, ql],
                                         xqr[co][:, ql])
                else:
                    nc.vector.tensor_add(ot[co][:, ql], u3[:], xqr[co][:, ql])
                # split the 128KB output transfers over the two HWDGE rings
                # so the trailing DMA after the last compute is halved
                ring = nc.sync if co == 0 else nc.scalar
                ring.dma_start(out_d[co * 128:(co + 1) * 128, ql],
                               ot[co][:, ql])

        # last-chunk drain + tail, pipelined over the two query halves
        drain_half(0, sc, el, e2i, spt_t)
        drain_half(1, sc, el, e2i, spt_t)
        tail_half(0)
        tail_half(1)
        pt_ctx.close()

    nc.compile()
    return nc


def _prep_inputs(inputs):
    import ml_dtypes
    E4 = ml_dtypes.float8_e4m3
    bf = lambda a: np.ascontiguousarray(
        np.asarray(a, dtype=np.float32).astype(ml_dtypes.bfloat16))
    f8 = lambda a: np.ascontiguousarray(
        np.asarray(a, dtype=np.float32).astype(E4))
    f = lambda a: np.ascontiguousarray(np.asarray(a, dtype=np.float32))
    planar = lambda a: np.ascontiguousarray(
        np.asarray(a, np.float32).reshape(2, 128, -1).transpose(1, 0, 2))

    x32 = f(inputs["corr_feat_belief"][0])                  # [D, N]
    spT = np.asarray(inputs["spatial_compatibility"][0]).T  # [N(keys), N(q)]
    Wq, bq = f(inputs["Wq"]), f(inputs["bq"])
    Wk, bk = f(inputs["Wk"]), f(inputs["bk"])
    Wv, bv = f(inputs["Wv"]), f(inputs["bv"])
    W1, b1, g1, be1 = f(inputs["W1"]), f(inputs["b1"]), f(inputs["g1"]), f(inputs["be1"])
    W2, b2, g2, be2 = f(inputs["W2"]), f(inputs["b2"]), f(inputs["g2"]), f(inputs["be2"])
    W3, b3 = f(inputs["W3"]), f(inputs["b3"])

    scale = np.float32(1.0 / np.sqrt(DH))
    s1 = (g1 / np.sqrt(np.float32(1.0) + np.float32(1e-5))).astype(np.float32)
    s2 = (g2 / np.sqrt(np.float32(1.0) + np.float32(1e-5))).astype(np.float32)

    xpl = planar(x32)               # [128, 2, N]; channel c = p + 128*j
    x8 = f8(xpl)
    # quarter-major so each quarter DMA reads contiguous 2KB/partition lines
    x8q = np.ascontiguousarray(
        np.stack([x8[:, :, k * 1024:(k + 1) * 1024] for k in range(4)]))
    spT_bf = bf(spT)
    common = dict(
        x8=x8q,
        wq8=f8(planar(Wq.T) * WS),
        wk8=f8(planar(Wk.T) * WS),
        wv8=f8(planar(Wv.T) * WS),
        w1t=bf((W1 * s1[:, None]).T),
        w2t=bf((W2 * s2[:, None]).T),
        w3t=bf(W3.T),
        bq2=f((bq * scale).reshape(2, 128).T),
        bk2=f(bk.reshape(2, 128).T),
        bv2=f(bv.reshape(2, 128).T),
        b1f=f((s1 * b1 + be1).reshape(128, 1)),
        b2f=f((s2 * b2 + be2).reshape(128, 1)),
        b32=f(b3.reshape(2, 128).T),
    )
    in_maps = []
    for m in range(NCORES):
        sl = slice(m * NQ, (m + 1) * NQ)
        im = dict(common)
        im["xq8"] = np.ascontiguousarray(x8[:, :, sl])
        im["xqr"] = f(x32[:, sl])
        # group 4 key chunks per DMA: [8, 128, 4*NQ] with 4KB partition lines
        im["spt"] = np.ascontiguousarray(
            spT_bf[:, sl].reshape(8, 4, 128, NQ).transpose(0, 2, 1, 3)
            .reshape(8, 128, 4 * NQ))
        in_maps.append(im)
    flags = tuple(bool(np.any(b != 0)) for b in (bq, bk, bv, b3))
    return in_maps, flags


def _run(inputs, trace=False):
    from concourse.bass_utils import run_bass_kernel_spmd
    in_maps, flags = _prep_inputs(inputs)
    if flags not in _CACHE:
        _CACHE[flags] = _build(*flags)
    nc = _CACHE[flags]
    res = run_bass_kernel_spmd(nc, in_maps, core_ids=list(range(NCORES)),
                               trace=trace)
    out = np.concatenate([res.results[m]["out"] for m in range(NCORES)],
                         axis=1)[None]
    return np.ascontiguousarray(out.astype(np.float32)), res


def kernel(**inputs):
    out, _ = _run(inputs, trace=False)
    return out


# revision 30
# speedup vs baseline: 1.2351x; 1.0241x over previous
"""Bass/Tile TRN2 kernel for a non-local attention block (BaseNonLocalBlock).

Contract: kernel(**inputs) takes the FULL inputs of the nn.Module problem
(B=1, D=256, H=4, N=4096) and returns the FULL output [1, 256, 4096].

Sharding: query columns of the N x N attention are split across the 8
NeuronCores (512 queries per core). K/V projections are computed
redundantly on every core (cheap); each core produces its own output
column slice and the host concatenates.

Per-core structure (flash-attention style, scores never hit HBM):
  pre-phase: Q/K/V conv1x1 projections as fp8 DoubleRow matmuls
    (channels packed planar [128, 2, *]; weights prescaled x16 on the
    host, un-scaled for free in the PSUM->SBUF copy scale).  K -> bf16
    per-block tiles; V^T(+ones col per head) -> bf16 per-block tiles.
    Startup DMAs are spread over the sync/scalar/gpsimd rings (one
    HWDGE queue sustains only ~60 GB/s); a ~4us block of dummy matmuls
    warms the PE HAM clock gate (1.2 -> 2.4 GHz) during the DMA ramp.
  loop over 32 key chunks (128 keys each):
    S_T = K_h[:, chunk]^T @ Q_h       (PSUM, 2 heads per row-split pair)
    sc  = copy(S_T)                   (ACT PSUM->SBUF bf16, 2x ~1.0us --
                                       ACT's only loop duty; EXP is gone)
    el  = spt * sc                    (DVE bf16 tensor_tensor at 2x mode,
                                       ~1.25us for all 4 heads)
    e2i = int16(el*184.665 + 16250.4) (DVE tensor_scalar at 4x, ~0.6us:
                                       Schraudolph bit trick -- the int16
                                       bits ARE bf16(exp(el) * 2^-c); the
                                       constant 2^-c factor cancels in the
                                       softmax normalization)
    msg_h += vt^T @ e2i.bitcast(bf16) (bf16 matmul per head, 1-2 iters
                                       behind; vt row 64 of ones
                                       accumulates the denominator)
  tail (pipelined over two query halves): denominators -> PE ones-
    broadcast -> reciprocal (split DVE approx / ACT table) -> per-head
    multiply -> conv MLP with BN folded into W1/W2 -> residual add.

The bit-trick exp replaces ACT's 2.0us/iter EXP with a 0.6us DVE op at
4x packing, and the PSUM-sourced DVE multiply (1x, 2.7us) becomes a
bf16 SBUF multiply (2x, 1.25us) fed by the ACT copy.  Engines balance
at ~2.1us/iter (PE ~2.1, ACT ~2.1, DVE ~1.9).  keep_warm() dummy
matmuls paper over early PE idle gaps so the HAM activity monitor never
re-throttles the clock mid-kernel.  Numerics: the trick's piecewise-
linear 2^frac has +-3% per-element error which largely cancels between
softmax numerator and denominator (and vt/e2 are now bf16, removing the
old fp8 quantization); end-to-end rel error ~1e-4 vs tolerance 2e-2.
"""
import numpy as np
from contextlib import ExitStack

D = 256
N = 4096
NQ = 512          # queries per core
H = 4
DH = 64
NCORES = 8
NIT = N // 128    # 32 key chunks
VTS = 68          # padded per-head stride in the V_T-aug tile
WS = 16.0         # host prescale on conv weights before fp8 quantization
EC1 = 184.6650    # 2^7 / ln(2): bf16 exponent scale for the exp bit trick
EC2 = 16250.4     # 127*2^7 minus half the interp error (Schraudolph magic)

_CACHE = {}


def _build(has_bq, has_bk, has_bv, has_b3):
    import concourse.bass as bass
    import concourse.tile as tile
    from concourse import bacc, mybir

    F32 = mybir.dt.float32
    BF16 = mybir.dt.bfloat16
    I16 = mybir.dt.int16
    FP8 = mybir.dt.float8e4
    Id = mybir.ActivationFunctionType.Identity
    Relu = mybir.ActivationFunctionType.Relu
    DR = mybir.MatmulPerfMode.DoubleRow
    Mul = mybir.AluOpType.mult
    Add = mybir.AluOpType.add

    nc = bacc.Bacc("TRN2", target_bir_lowering=False, debug=False,
                   num_devices=NCORES)

    # DRAM I/O (per core)
    x8_d = nc.dram_tensor("x8", [4, 128, 2, N // 4], FP8,
                          kind="ExternalInput").ap()
    xq8_d = nc.dram_tensor("xq8", [128, 2, NQ], FP8, kind="ExternalInput").ap()
    xqr_d = nc.dram_tensor("xqr", [D, NQ], F32, kind="ExternalInput").ap()
    # spt grouped host-side: [8 groups, 128 partitions, 4 chunks * NQ] so one
    # 512KB dma_start covers 4 key chunks with 4KB contiguous partition lines
    # (the ~2us fixed DMA cost is amortized 4x vs per-chunk transfers)
    spt_d = nc.dram_tensor("spt", [8, 128, 4 * NQ], BF16,
                           kind="ExternalInput").ap()
    wq8_d = nc.dram_tensor("wq8", [128, 2, D], FP8, kind="ExternalInput").ap()
    wk8_d = nc.dram_tensor("wk8", [128, 2, D], FP8, kind="ExternalInput").ap()
    wv8_d = nc.dram_tensor("wv8", [128, 2, D], FP8, kind="ExternalInput").ap()
    w1t_d = nc.dram_tensor("w1t", [D, 128], BF16, kind="ExternalInput").ap()
    w2t_d = nc.dram_tensor("w2t", [128, 128], BF16, kind="ExternalInput").ap()
    w3t_d = nc.dram_tensor("w3t", [128, D], BF16, kind="ExternalInput").ap()
    bq_d = nc.dram_tensor("bq2", [128, 2], F32, kind="ExternalInput").ap()
    bk_d = nc.dram_tensor("bk2", [128, 2], F32, kind="ExternalInput").ap()
    bv_d = nc.dram_tensor("bv2", [128, 2], F32, kind="ExternalInput").ap()
    b1_d = nc.dram_tensor("b1f", [128, 1], F32, kind="ExternalInput").ap()
    b2_d = nc.dram_tensor("b2f", [128, 1], F32, kind="ExternalInput").ap()
    b3_d = nc.dram_tensor("b32", [128, 2], F32, kind="ExternalInput").ap()
    out_d = nc.dram_tensor("out", [D, NQ], F32, kind="ExternalOutput").ap()

    with tile.TileContext(nc) as tc, ExitStack() as ctx:
        sb = ctx.enter_context(tc.tile_pool(name="sb", bufs=1))
        spt_pool = ctx.enter_context(tc.tile_pool(name="sptp", bufs=3))
        sc_pool = ctx.enter_context(tc.tile_pool(name="scp", bufs=3))
        el_pool = ctx.enter_context(tc.tile_pool(name="elp", bufs=3))
        e2_pool = ctx.enter_context(tc.tile_pool(name="e2p", bufs=4))
        pj_ctx = ExitStack()
        pj = pj_ctx.enter_context(tc.tile_pool(name="pj", bufs=4, space="PSUM"))

        # ---- early inputs. One HWDGE queue sustains only ~60 GB/s, so the
        # startup transfers are spread over the three rings: sync takes the
        # x quarters, scalar the small weight tensors plus half of quarter 0,
        # gpsimd the other half before its spt stream. ----
        # all bulk DMA rides the two HWDGE rings; the gpsimd SWDGE ring is
        # left unused (its descriptor-gen and sem teardown cost more)
        # the small weight tensors go FIRST on the scalar ring -- the first
        # K projection needs wk8, and anything queued ahead of it delays the
        # whole pre-phase
        wq8 = sb.tile([128, 2, D], FP8, name="wq8")
        wk8 = sb.tile([128, 2, D], FP8, name="wk8")
        wv8 = sb.tile([128, 2, D], FP8, name="wv8")
        nc.scalar.dma_start(wk8[:], wk8_d[:, :, :])
        nc.scalar.dma_start(wv8[:], wv8_d[:, :, :])
        xq8 = sb.tile([128, 2, NQ], FP8, name="xq8")
        nc.scalar.dma_start(xq8[:], xq8_d[:, :, :])
        nc.scalar.dma_start(wq8[:], wq8_d[:, :, :])
        x8t = [sb.tile([128, 2, 1024], FP8, name=f"x8_{k}") for k in range(4)]
        for k in (0, 1):
            nc.sync.dma_start(x8t[k][:], x8_d[k])
        for k in (2, 3):
            nc.scalar.dma_start(x8t[k][:], x8_d[k])
        if has_bq:
            bq = sb.tile([128, 2], F32, name="bq")
            nc.scalar.dma_start(bq[:], bq_d[:, :])
        if has_bk:
            bk = sb.tile([128, 2], F32, name="bk")
            nc.scalar.dma_start(bk[:], bk_d[:, :])

        k_sb = [sb.tile([128, 2, NQ], BF16, name=f"ksb{ib}") for ib in range(8)]
        q_sb = [sb.tile([128, NQ], BF16, name=f"q{co}") for co in range(2)]
        # V^T augmented: per key-chunk it, per head h: [64 V cols | ones | pad]
        vt = [sb.tile([128, 4, H, VTS], BF16, name=f"vt{ib}") for ib in range(8)]
        for ib in range(8):
            nc.gpsimd.memset(vt[ib][:, :, :, 64:65], 1.0)
        ones64 = sb.tile([1, 64], BF16, name="ones64")
        nc.gpsimd.memset(ones64[:], 1.0)

        # ---- PE warmup: ~3.5us of tiny matmuls during the DMA ramp so the
        # HAM clock gate is already at 8/8 when real projections start ----
        warm = sb.tile([128, NQ], BF16, name="warm")
        nc.vector.memset(warm[:].bitcast(F32)[:, 0:256], 0.0)
        wps = pj.tile([128, NQ], F32, tag="t")
        for r in range(6):
            nc.tensor.matmul(wps[0:64, :], warm[:, 0:64], warm[:],
                             start=True, stop=True)

        def keep_warm(ap, n):
            # dummy matmuls into a PSUM region that a later start=True matmul
            # fully overwrites; fills PE idle gaps so the HAM clock stays 8/8
            for r in range(n):
                nc.tensor.matmul(ap, warm[:, 0:64], warm[:, 0:64],
                                 start=True, stop=True)

        # spt group prefetch: all on the sync ring, which is otherwise idle
        # during the loop (one 512KB group every ~3.2us vs ~8.2us consumption)
        spt_groups = {}

        def load_spt_group(g):
            t = spt_pool.tile([128, 4, NQ], BF16, tag="spt")
            nc.sync.dma_start(t[:].rearrange("p t o -> p (t o)"), spt_d[g])
            spt_groups[g] = t

        for g in range(2):
            load_spt_group(g)

        def q_proj():
            # fp8 DoubleRow conv1x1, contraction = 256 channels
            for co in range(2):
                ps = pj.tile([128, NQ], F32, tag="t")
                nc.tensor.matmul(ps[:], wq8[:, :, co * 128:(co + 1) * 128],
                                 xq8[:], start=True, stop=True, perf_mode=DR)
                if has_bq:
                    nc.scalar.activation(q_sb[co][:], ps[:], Id,
                                         scale=1.0 / (WS * 8.0),
                                         bias=bq[:, co:co + 1])
                else:
                    nc.scalar.activation(q_sb[co][:], ps[:], Id,
                                         scale=1.0 / (WS * 8.0))

        # ---- K / V^T projections per 512-key block, copies chase on
        # alternating ACT/DVE ----
        cp = [0]

        def copy_scaled(dst, src, bias=None):
            if bias is not None:
                nc.scalar.activation(dst, src, Id, scale=1.0 / WS, bias=bias)
            elif cp[0] % 2 == 0:
                nc.scalar.activation(dst, src, Id, scale=1.0 / WS)
            else:
                nc.vector.tensor_scalar_mul(dst, src, 1.0 / WS)
            cp[0] += 1

        for ib in range(8):
            xt = x8t[ib // 2]
            xo = (ib % 2) * 512
            kps = pj.tile([128, 2, NQ], F32, tag="t")
            for co in range(2):
                nc.tensor.matmul(kps[:, co, :],
                                 wk8[:, :, co * 128:(co + 1) * 128],
                                 xt[:, :, xo:xo + 512],
                                 start=True, stop=True, perf_mode=DR)
            ksl = k_sb[ib][:]
            if has_bk:
                for co in range(2):
                    nc.scalar.activation(ksl[:, co, :], kps[:, co, :], Id,
                                         scale=1.0 / WS, bias=bk[:, co:co + 1])
            else:
                copy_scaled(ksl, kps[:])
            vps = pj.tile([128, 4, 256], F32, tag="t")
            for u in range(4):
                ko = xo + u * 128
                nc.tensor.matmul(vps[:, u, 0:D], xt[:, :, ko:ko + 128],
                                 wv8[:], start=True, stop=True,
                                 perf_mode=DR)
            vdst = vt[ib][:, 0:4, :, 0:64]
            vsrc = vps[:, 0:4, 0:D].rearrange("p w (h c) -> p w h c", h=H)
            copy_scaled(vdst, vsrc)
        # q is only needed by the loop itself; projecting it last keeps the
        # pre-phase PSUM slot rotation from serializing on the q tiles
        q_proj()
        wps2 = pj.tile([128, NQ], F32, tag="t")
        keep_warm(wps2[0:64, 0:64], 4)

        pj_ctx.close()
        ps_m = ctx.enter_context(tc.tile_pool(name="psm", bufs=1, space="PSUM"))
        pst_ctx = ExitStack()
        ps_t = pst_ctx.enter_context(
            tc.tile_pool(name="pst", bufs=2, space="PSUM"))
        # messages for all 4 heads in one 4-bank PSUM tensor; row 64 of each
        # bank accumulates the softmax denominator (ones column in vt)
        mps = ps_m.tile([65, H, NQ], F32, name="mps")

        def head_bcast(ap, w):
            # broadcast a [128, q] AP over the head axis with a 0-stride dim
            return bass.AP(tensor=ap.tensor, offset=ap.offset,
                           ap=[list(ap.ap[0]), [0, w], list(ap.ap[1])])

        # ---- attention loop ----
        pendq = []

        def emit_msgs(p):
            pit, e2p = p
            e2b = e2p[:].bitcast(BF16)
            for h in range(H):
                nc.tensor.matmul(mps[:, h, :], vt[pit // 4][:, pit % 4, h, 0:65],
                                 e2b[:, h, :],
                                 start=(pit == 0), stop=(pit == NIT - 1),
                                 skip_group_check=True)

        for it in range(NIT):
            if it % 4 == 0 and it // 4 + 2 < 8:
                load_spt_group(it // 4 + 2)
            spt_t = spt_groups[it // 4][:, it % 4, :]
            sc = sc_pool.tile([128, H, NQ], BF16, tag="sc")
            el = el_pool.tile([128, H, NQ], BF16, tag="el")
            e2i = e2_pool.tile([128, H, NQ], I16, tag="e2")
            sps_l = []
            for hp in range(2):
                sps = ps_t.tile([128, 2, NQ], F32, tag="t")
                sps_l.append(sps)
                if it < 2:
                    keep_warm(sps[0:64, 0, 0:64], 8)
                elif it < 4:
                    keep_warm(sps[0:64, 0, 0:64], 3)
                for jj in range(2):
                    ro = jj * 64
                    nc.tensor.matmul(
                        sps[:, jj, :],
                        k_sb[it // 4][ro:ro + 64, hp,
                                      (it % 4) * 128:(it % 4) * 128 + 128],
                        q_sb[hp][ro:ro + 64, :],
                        start=True, stop=True)
                nc.scalar.copy(sc[:, 2 * hp:2 * hp + 2, :], sps[:])
            if it < NIT - 1:
                nc.vector.tensor_mul(el[:], sc[:], head_bcast(spt_t, H))
                nc.vector.tensor_scalar(e2i[:], el[:], EC1, EC2, Mul, Add)
                # messages run TWO iterations behind so the producer chain
                # (ACT copy -> DVE TT -> DVE TS) is finished when PE arrives
                if len(pendq) >= 2:
                    emit_msgs(pendq.pop(0))
                pendq.append((it, e2i))
            else:
                # final iteration: drain pending messages, then pipeline the
                # last chunk + tail over two query halves
                for p in pendq:
                    emit_msgs(p)
                pendq = []

        pst_ctx.close()

        # ---- late inputs (only needed after the attention loop) ----
        # per-head row slices of W1^T at base partition 0 (PE requires lhsT
        # and rhs to share the base partition).  All late inputs ride the
        # sync ring: its sequencer is idle by now, while a DIRECT2D on the
        # scalar ring would stall the ACT datapath right at tail start.
        w1t4 = sb.tile([64, H, 128], BF16, name="w1t4")
        nc.sync.dma_start(w1t4[:], w1t_d.rearrange("(h p) o -> p h o", p=64))
        w2t = sb.tile([128, 128], BF16, name="w2t")
        nc.sync.dma_start(w2t[:], w2t_d[:, :])
        w3t = sb.tile([128, D], BF16, name="w3t")
        nc.sync.dma_start(w3t[:], w3t_d[:, :])
        xqr = [sb.tile([128, NQ], F32, name=f"xqr{co}") for co in range(2)]
        for co in range(2):
            nc.sync.dma_start(xqr[co][:], xqr_d[co * 128:(co + 1) * 128, :])
        b1 = sb.tile([128, 1], F32, name="b1")
        b2 = sb.tile([128, 1], F32, name="b2")
        nc.sync.dma_start(b1[:], b1_d[:, :])
        nc.sync.dma_start(b2[:], b2_d[:, :])
        if has_bv:
            bv = sb.tile([128, 2], F32, name="bv")
            nc.sync.dma_start(bv[:], bv_d[:, :])
        if has_b3:
            b3 = sb.tile([128, 2], F32, name="b3")
            nc.sync.dma_start(b3[:], b3_d[:, :])

        # tail tiles
        HQ = NQ // 2
        dhs = sb.tile([1, H, NQ], BF16, name="dhs")
        msg4 = sb.tile([64, H, NQ], BF16, name="msg4")
        rbc = sb.tile([64, H, NQ], F32, name="rbc")
        h1 = sb.tile([128, NQ], BF16, name="h1")
        h2 = sb.tile([128, NQ], BF16, name="h2")
        ot = [sb.tile([128, NQ], F32, name=f"ot{co}") for co in range(2)]
        tb = sb.tile([128, NQ], F32, name="tb")

        pt_ctx = ExitStack()
        pd = pt_ctx.enter_context(tc.tile_pool(name="pd", bufs=1, space="PSUM"))
        pt = pt_ctx.enter_context(tc.tile_pool(name="pt", bufs=2, space="PSUM"))

        def drain_half(qh, sc, el, e2i, spt_t):
            ql = slice(qh * HQ, (qh + 1) * HQ)
            nc.vector.tensor_mul(el[:, :, ql], sc[:, :, ql],
                                 head_bcast(spt_t[:, ql], H))
            nc.vector.tensor_scalar(e2i[:, :, ql], el[:, :, ql], EC1, EC2,
                                    Mul, Add)
            e2b = e2i[:].bitcast(BF16)
            for h in range(H):
                nc.tensor.matmul(mps[:, h, ql], vt[7][:, 3, h, 0:65],
                                 e2b[:, h, ql],
                                 start=False, stop=True,
                                 skip_group_check=True)

        def tail_norm(qh):
            # denominators -> PE ones-broadcast -> reciprocal -> per-head
            # multiply.  Emitted for BOTH halves before any MLP work so the
            # second half's chain is not queued behind the first half's MLP.
            ql = slice(qh * HQ, (qh + 1) * HQ)
            # both denominator copies on ACT: the DVE is the tail bottleneck
            nc.scalar.copy(dhs[:, 0:2, ql], mps[64:65, 0:2, ql])
            nc.scalar.copy(dhs[:, 2:4, ql], mps[64:65, 2:4, ql])
            dbb = pd.tile([64, H, HQ], F32, tag="d")
            for hp in range(2):
                nc.tensor.matmul(dbb[:, 2 * hp:2 * hp + 2, :], ones64[:],
                                 dhs[:, 2 * hp:2 * hp + 2, ql],
                                 start=True, stop=True)
            nc.vector.reciprocal_approx_fast(out=rbc[:, :, ql], in_=dbb[:])
            # normalize 2 heads per DVE op; the W1 matmul contracts each
            # head's 64 channels from its own row slice of w1t
            for co in range(2):
                nc.vector.tensor_mul(msg4[:, 2 * co:2 * co + 2, ql],
                                     mps[0:64, 2 * co:2 * co + 2, ql],
                                     rbc[:, 2 * co:2 * co + 2, ql])
            if has_bv:
                for h in range(H):
                    ro = (h % 2) * 64
                    nc.scalar.activation(msg4[:, h, ql], msg4[:, h, ql], Id,
                                         bias=bv[ro:ro + 64, h // 2:h // 2 + 1])

        def tail_mlp(qh):
            ql = slice(qh * HQ, (qh + 1) * HQ)
            u1 = pt.tile([128, HQ], F32, tag="t")
            for h in range(H):
                nc.tensor.matmul(u1[:], w1t4[:, h, :], msg4[:, h, ql],
                                 start=(h == 0), stop=(h == H - 1))
            nc.scalar.activation(h1[:, ql], u1[:], Relu, bias=b1[:, 0:1])
            u2 = pt.tile([128, HQ], F32, tag="t")
            nc.tensor.matmul(u2[:], w2t[:], h1[:, ql], start=True, stop=True)
            nc.scalar.activation(h2[:, ql], u2[:], Relu, bias=b2[:, 0:1])
            for co in range(2):
                u3 = pt.tile([128, HQ], F32, tag="t")
                nc.tensor.matmul(u3[:], w3t[:, co * 128:(co + 1) * 128],
                                 h2[:, ql], start=True, stop=True)
                if has_b3:
                    nc.scalar.activation(tb[:, ql], u3[:], Id,
                                         bias=b3[:, co:co + 1])
                    nc.vector.tensor_add(ot[co][:, ql], tb[:, ql],
                                         xqr[co][:, ql])
                else:
                    nc.vector.tensor_add(ot[co][:, ql], u3[:], xqr[co][:, ql])
                # split the 128KB output transfers over the two HWDGE rings
                # so the trailing DMA after the last compute is halved
                ring = nc.sync if co == 0 else nc.scalar
                ring.dma_start(out_d[co * 128:(co + 1) * 128, ql],
                               ot[co][:, ql])

        # last-chunk drain + tail, pipelined over the two query halves
        drain_half(0, sc, el, e2i, spt_t)
        drain_half(1, sc, el, e2i, spt_t)
        tail_norm(0)
        tail_norm(1)
        tail_mlp(0)
        tail_mlp(1)
        pt_ctx.close()

    nc.compile()
    return nc


def _prep_inputs(inputs):
    import ml_dtypes
    E4 = ml_dtypes.float8_e4m3
    bf = lambda a: np.ascontiguousarray(
        np.asarray(a, dtype=np.float32).astype(ml_dtypes.bfloat16))
    f8 = lambda a: np.ascontiguousarray(
        np.asarray(a, dtype=np.float32).astype(E4))
    f = lambda a: np.ascontiguousarray(np.asarray(a, dtype=np.float32))
    planar = lambda a: np.ascontiguousarray(
        np.asarray(a, np.float32).reshape(2, 128, -1).transpose(1, 0, 2))

    x32 = f(inputs["corr_feat_belief"][0])                  # [D, N]
    spT = np.asarray(inputs["spatial_compatibility"][0]).T  # [N(keys), N(q)]
    Wq, bq = f(inputs["Wq"]), f(inputs["bq"])
    Wk, bk = f(inputs["Wk"]), f(inputs["bk"])
    Wv, bv = f(inputs["Wv"]), f(inputs["bv"])
    W1, b1, g1, be1 = f(inputs["W1"]), f(inputs["b1"]), f(inputs["g1"]), f(inputs["be1"])
    W2, b2, g2, be2 = f(inputs["W2"]), f(inputs["b2"]), f(inputs["g2"]), f(inputs["be2"])
    W3, b3 = f(inputs["W3"]), f(inputs["b3"])

    scale = np.float32(1.0 / np.sqrt(DH))
    s1 = (g1 / np.sqrt(np.float32(1.0) + np.float32(1e-5))).astype(np.float32)
    s2 = (g2 / np.sqrt(np.float32(1.0) + np.float32(1e-5))).astype(np.float32)

    xpl = planar(x32)               # [128, 2, N]; channel c = p + 128*j
    x8 = f8(xpl)
    # quarter-major so each quarter DMA reads contiguous 2KB/partition lines
    x8q = np.ascontiguousarray(
        np.stack([x8[:, :, k * 1024:(k + 1) * 1024] for k in range(4)]))
    spT_bf = bf(spT)
    common = dict(
        x8=x8q,
        wq8=f8(planar(Wq.T) * WS),
        wk8=f8(planar(Wk.T) * WS),
        wv8=f8(planar(Wv.T) * WS),
        w1t=bf((W1 * s1[:, None]).T),
        w2t=bf((W2 * s2[:, None]).T),
        w3t=bf(W3.T),
        bq2=f((bq * scale).reshape(2, 128).T),
        bk2=f(bk.reshape(2, 128).T),
        bv2=f(bv.reshape(2, 128).T),
        b1f=f((s1 * b1 + be1).reshape(128, 1)),
        b2f=f((s2 * b2 + be2).reshape(128, 1)),
        b32=f(b3.reshape(2, 128).T),
    )
    in_maps = []
    for m in range(NCORES):
        sl = slice(m * NQ, (m + 1) * NQ)
        im = dict(common)
        im["xq8"] = np.ascontiguousarray(x8[:, :, sl])
        im["xqr"] = f(x32[:, sl])
        # group 4 key chunks per DMA: [8, 128, 4*NQ] with 4KB partition lines
        im["spt"] = np.ascontiguousarray(
            spT_bf[:, sl].reshape(8, 4, 128, NQ).transpose(0, 2, 1, 3)
            .reshape(8, 128, 4 * NQ))
        in_maps.append(im)
    flags = tuple(bool(np.any(b != 0)) for b in (bq, bk, bv, b3))
    return in_maps, flags


def _run(inputs, trace=False):
    from concourse.bass_utils import run_bass_kernel_spmd
    in_maps, flags = _prep_inputs(inputs)
    if flags not in _CACHE:
        _CACHE[flags] = _build(*flags)
    nc = _CACHE[flags]
    res = run_bass_kernel_spmd(nc, in_maps, core_ids=list(range(NCORES)),
                               trace=trace)
    out = np.concatenate([res.results[m]["out"] for m in range(NCORES)],
                         axis=1)[None]
    return np.ascontiguousarray(out.astype(np.float32)), res


def kernel(**inputs):
    out, _ = _run(inputs, trace=False)
    return out


# revision 33
# speedup vs baseline: 1.2774x; 1.0342x over previous
"""Bass/Tile TRN2 kernel for a non-local attention block (BaseNonLocalBlock).

Contract: kernel(**inputs) takes the FULL inputs of the nn.Module problem
(B=1, D=256, H=4, N=4096) and returns the FULL output [1, 256, 4096].

Sharding: query columns of the N x N attention are split across the 8
NeuronCores (512 queries per core). K/V projections are computed
redundantly on every core (cheap); each core produces its own output
column slice and the host concatenates.

Per-core structure (flash-attention style, scores never hit HBM):
  pre-phase: Q/K/V conv1x1 projections as fp8 DoubleRow matmuls
    (channels packed planar [128, 2, *]; weights prescaled x16 on the
    host, un-scaled for free in the PSUM->SBUF copy scale).  K -> bf16
    per-block tiles; V^T(+ones col per head) -> bf16 per-block tiles.
    Startup DMAs are spread over the sync/scalar/gpsimd rings (one
    HWDGE queue sustains only ~60 GB/s); a ~4us block of dummy matmuls
    warms the PE HAM clock gate (1.2 -> 2.4 GHz) during the DMA ramp.
  loop over 32 key chunks (128 keys each):
    S_T = K_h[:, chunk]^T @ Q_h       (PSUM, 2 heads per row-split pair)
    sc  = copy(S_T)                   (ACT PSUM->SBUF bf16, 2x ~1.0us --
                                       ACT's only loop duty; EXP is gone)
    el  = spt * sc                    (DVE bf16 tensor_tensor at 2x mode,
                                       ~1.25us for all 4 heads)
    e2i = int16(el*184.665 + 16250.4) (DVE tensor_scalar at 4x, ~0.6us:
                                       Schraudolph bit trick -- the int16
                                       bits ARE bf16(exp(el) * 2^-c); the
                                       constant 2^-c factor cancels in the
                                       softmax normalization)
    msg_h += vt^T @ e2i.bitcast(bf16) (bf16 matmul per head, 1-2 iters
                                       behind; vt row 64 of ones
                                       accumulates the denominator)
  tail (pipelined over two query halves): denominators -> PE ones-
    broadcast -> reciprocal (split DVE approx / ACT table) -> per-head
    multiply -> conv MLP with BN folded into W1/W2 -> residual add.

The bit-trick exp replaces ACT's 2.0us/iter EXP with a 0.6us DVE op at
4x packing, and the PSUM-sourced DVE multiply (1x, 2.7us) becomes a
bf16 SBUF multiply (2x, 1.25us) fed by the ACT copy.  Engines balance
at ~2.1us/iter (PE ~2.1, ACT ~2.1, DVE ~1.9).  keep_warm() dummy
matmuls paper over early PE idle gaps so the HAM activity monitor never
re-throttles the clock mid-kernel.  Numerics: the trick's piecewise-
linear 2^frac has +-3% per-element error which largely cancels between
softmax numerator and denominator (and vt/e2 are now bf16, removing the
old fp8 quantization); end-to-end rel error ~1e-4 vs tolerance 2e-2.
"""
import numpy as np
from contextlib import ExitStack

D = 256
N = 4096
NQ = 512          # queries per core
H = 4
DH = 64
NCORES = 8
NIT = N // 128    # 32 key chunks
VTS = 68          # padded per-head stride in the V_T-aug tile
WS = 16.0         # host prescale on conv weights before fp8 quantization
EC1 = 184.6650    # 2^7 / ln(2): bf16 exponent scale for the exp bit trick
EC2 = 16250.4     # 127*2^7 minus half the interp error (Schraudolph magic)

_CACHE = {}


def _build(has_bq, has_bk, has_bv, has_b3):
    import concourse.bass as bass
    import concourse.tile as tile
    from concourse import bacc, mybir

    F32 = mybir.dt.float32
    BF16 = mybir.dt.bfloat16
    I16 = mybir.dt.int16
    FP8 = mybir.dt.float8e4
    Id = mybir.ActivationFunctionType.Identity
    Relu = mybir.ActivationFunctionType.Relu
    DR = mybir.MatmulPerfMode.DoubleRow
    Mul = mybir.AluOpType.mult
    Add = mybir.AluOpType.add

    nc = bacc.Bacc("TRN2", target_bir_lowering=False, debug=False,
                   num_devices=NCORES)

    # DRAM I/O (per core)
    x8_d = nc.dram_tensor("x8", [4, 128, 2, N // 4], FP8,
                          kind="ExternalInput").ap()
    xq8_d = nc.dram_tensor("xq8", [128, 2, NQ], FP8, kind="ExternalInput").ap()
    xqr_d = nc.dram_tensor("xqr", [D, NQ], F32, kind="ExternalInput").ap()
    # spt grouped host-side: [8 groups, 128 partitions, 4 chunks * NQ] so one
    # 512KB dma_start covers 4 key chunks with 4KB contiguous partition lines
    # (the ~2us fixed DMA cost is amortized 4x vs per-chunk transfers)
    spt_d = nc.dram_tensor("spt", [8, 128, 4 * NQ], BF16,
                           kind="ExternalInput").ap()
    wq8_d = nc.dram_tensor("wq8", [128, 2, D], FP8, kind="ExternalInput").ap()
    wk8_d = nc.dram_tensor("wk8", [128, 2, D], FP8, kind="ExternalInput").ap()
    wv8_d = nc.dram_tensor("wv8", [128, 2, D], FP8, kind="ExternalInput").ap()
    w1t_d = nc.dram_tensor("w1t", [D, 128], BF16, kind="ExternalInput").ap()
    w2t_d = nc.dram_tensor("w2t", [128, 128], BF16, kind="ExternalInput").ap()
    w3t_d = nc.dram_tensor("w3t", [128, D], BF16, kind="ExternalInput").ap()
    bq_d = nc.dram_tensor("bq2", [128, 2], F32, kind="ExternalInput").ap()
    bk_d = nc.dram_tensor("bk2", [128, 2], F32, kind="ExternalInput").ap()
    bv_d = nc.dram_tensor("bv2", [128, 2], F32, kind="ExternalInput").ap()
    b1_d = nc.dram_tensor("b1f", [128, 1], F32, kind="ExternalInput").ap()
    b2_d = nc.dram_tensor("b2f", [128, 1], F32, kind="ExternalInput").ap()
    b3_d = nc.dram_tensor("b32", [128, 2], F32, kind="ExternalInput").ap()
    out_d = nc.dram_tensor("out", [D, NQ], F32, kind="ExternalOutput").ap()

    with tile.TileContext(nc) as tc, ExitStack() as ctx:
        sb = ctx.enter_context(tc.tile_pool(name="sb", bufs=1))
        spt_pool = ctx.enter_context(tc.tile_pool(name="sptp", bufs=3))
        sc_pool = ctx.enter_context(tc.tile_pool(name="scp", bufs=3))
        el_pool = ctx.enter_context(tc.tile_pool(name="elp", bufs=3))
        e2_pool = ctx.enter_context(tc.tile_pool(name="e2p", bufs=4))
        pj_ctx = ExitStack()
        pj = pj_ctx.enter_context(tc.tile_pool(name="pj", bufs=4, space="PSUM"))

        # ---- early inputs. One HWDGE queue sustains only ~60 GB/s, so the
        # startup transfers are spread over the three rings: sync takes the
        # x quarters, scalar the small weight tensors plus half of quarter 0,
        # gpsimd the other half before its spt stream. ----
        # all bulk DMA rides the two HWDGE rings; the gpsimd SWDGE ring is
        # left unused (its descriptor-gen and sem teardown cost more)
        # the small weight tensors go FIRST on the scalar ring -- the first
        # K projection needs wk8, and anything queued ahead of it delays the
        # whole pre-phase
        wq8 = sb.tile([128, 2, D], FP8, name="wq8")
        wk8 = sb.tile([128, 2, D], FP8, name="wk8")
        wv8 = sb.tile([128, 2, D], FP8, name="wv8")
        nc.scalar.dma_start(wk8[:], wk8_d[:, :, :])
        nc.scalar.dma_start(wv8[:], wv8_d[:, :, :])
        xq8 = sb.tile([128, 2, NQ], FP8, name="xq8")
        nc.scalar.dma_start(xq8[:], xq8_d[:, :, :])
        nc.scalar.dma_start(wq8[:], wq8_d[:, :, :])
        # interleave quarters across the rings so consecutive blocks' inputs
        # arrive in projection order (sync: q0,q2; scalar: q1,q3 after the
        # small weight transfers)
        x8t = [sb.tile([128, 2, 1024], FP8, name=f"x8_{k}") for k in range(4)]
        for k in (0, 2):
            nc.sync.dma_start(x8t[k][:], x8_d[k])
        for k in (1, 3):
            nc.scalar.dma_start(x8t[k][:], x8_d[k])
        if has_bq:
            bq = sb.tile([128, 2], F32, name="bq")
            nc.scalar.dma_start(bq[:], bq_d[:, :])
        if has_bk:
            bk = sb.tile([128, 2], F32, name="bk")
            nc.scalar.dma_start(bk[:], bk_d[:, :])

        k_sb = [sb.tile([128, 2, NQ], BF16, name=f"ksb{ib}") for ib in range(8)]
        q_sb = [sb.tile([128, NQ], BF16, name=f"q{co}") for co in range(2)]
        # V^T augmented: per key-chunk it, per head h: [64 V cols | ones | pad]
        vt = [sb.tile([128, 4, H, VTS], BF16, name=f"vt{ib}") for ib in range(8)]
        for ib in range(8):
            nc.gpsimd.memset(vt[ib][:, :, :, 64:65], 1.0)
        ones64 = sb.tile([1, 64], BF16, name="ones64")
        nc.gpsimd.memset(ones64[:], 1.0)

        # ---- PE warmup: ~3.5us of tiny matmuls during the DMA ramp so the
        # HAM clock gate is already at 8/8 when real projections start ----
        warm = sb.tile([128, NQ], BF16, name="warm")
        nc.vector.memset(warm[:].bitcast(F32)[:, 0:256], 0.0)
        wps = pj.tile([128, NQ], F32, tag="t")
        for r in range(6):
            nc.tensor.matmul(wps[0:64, :], warm[:, 0:64], warm[:],
                             start=True, stop=True)

        def keep_warm(ap, n):
            # dummy matmuls into a PSUM region that a later start=True matmul
            # fully overwrites; fills PE idle gaps so the HAM clock stays 8/8
            for r in range(n):
                nc.tensor.matmul(ap, warm[:, 0:64], warm[:, 0:64],
                                 start=True, stop=True)

        # spt group prefetch: all on the sync ring, which is otherwise idle
        # during the loop (one 512KB group every ~3.2us vs ~8.2us consumption)
        spt_groups = {}

        def load_spt_group(g):
            t = spt_pool.tile([128, 4, NQ], BF16, tag="spt")
            nc.sync.dma_start(t[:].rearrange("p t o -> p (t o)"), spt_d[g])
            spt_groups[g] = t

        for g in range(2):
            load_spt_group(g)

        def q_proj():
            # fp8 DoubleRow conv1x1, contraction = 256 channels
            for co in range(2):
                ps = pj.tile([128, NQ], F32, tag="t")
                nc.tensor.matmul(ps[:], wq8[:, :, co * 128:(co + 1) * 128],
                                 xq8[:], start=True, stop=True, perf_mode=DR)
                if has_bq:
                    nc.scalar.activation(q_sb[co][:], ps[:], Id,
                                         scale=1.0 / (WS * 8.0),
                                         bias=bq[:, co:co + 1])
                else:
                    nc.scalar.activation(q_sb[co][:], ps[:], Id,
                                         scale=1.0 / (WS * 8.0))

        # ---- K / V^T projections per 512-key block, copies chase on
        # alternating ACT/DVE ----
        cp = [0]

        def copy_scaled(dst, src, bias=None):
            if bias is not None:
                nc.scalar.activation(dst, src, Id, scale=1.0 / WS, bias=bias)
            elif cp[0] % 2 == 0:
                nc.scalar.activation(dst, src, Id, scale=1.0 / WS)
            else:
                nc.vector.tensor_scalar_mul(dst, src, 1.0 / WS)
            cp[0] += 1

        for ib in range(8):
            xt = x8t[ib // 2]
            xo = (ib % 2) * 512
            kps = pj.tile([128, 2, NQ], F32, tag="t")
            for co in range(2):
                nc.tensor.matmul(kps[:, co, :],
                                 wk8[:, :, co * 128:(co + 1) * 128],
                                 xt[:, :, xo:xo + 512],
                                 start=True, stop=True, perf_mode=DR)
            ksl = k_sb[ib][:]
            if has_bk:
                for co in range(2):
                    nc.scalar.activation(ksl[:, co, :], kps[:, co, :], Id,
                                         scale=1.0 / WS, bias=bk[:, co:co + 1])
            else:
                copy_scaled(ksl, kps[:])
            vps = pj.tile([128, 4, 256], F32, tag="t")
            for u in range(4):
                ko = xo + u * 128
                nc.tensor.matmul(vps[:, u, 0:D], xt[:, :, ko:ko + 128],
                                 wv8[:], start=True, stop=True,
                                 perf_mode=DR)
            vdst = vt[ib][:, 0:4, :, 0:64]
            vsrc = vps[:, 0:4, 0:D].rearrange("p w (h c) -> p w h c", h=H)
            copy_scaled(vdst, vsrc)
        # q is only needed by the loop itself; projecting it last keeps the
        # pre-phase PSUM slot rotation from serializing on the q tiles
        q_proj()
        wps2 = pj.tile([128, NQ], F32, tag="t")
        keep_warm(wps2[0:64, 0:64], 4)

        pj_ctx.close()
        ps_m = ctx.enter_context(tc.tile_pool(name="psm", bufs=1, space="PSUM"))
        pst_ctx = ExitStack()
        ps_t = pst_ctx.enter_context(
            tc.tile_pool(name="pst", bufs=2, space="PSUM"))
        # messages for all 4 heads in one 4-bank PSUM tensor; row 64 of each
        # bank accumulates the softmax denominator (ones column in vt)
        mps = ps_m.tile([65, H, NQ], F32, name="mps")

        def head_bcast(ap, w):
            # broadcast a [128, q] AP over the head axis with a 0-stride dim
            return bass.AP(tensor=ap.tensor, offset=ap.offset,
                           ap=[list(ap.ap[0]), [0, w], list(ap.ap[1])])

        # ---- attention loop ----
        pendq = []

        def emit_msgs(p):
            pit, e2p = p
            e2b = e2p[:].bitcast(BF16)
            for h in range(H):
                nc.tensor.matmul(mps[:, h, :], vt[pit // 4][:, pit % 4, h, 0:65],
                                 e2b[:, h, :],
                                 start=(pit == 0), stop=(pit == NIT - 1),
                                 skip_group_check=True)

        for it in range(NIT):
            if it % 4 == 0 and it // 4 + 2 < 8:
                load_spt_group(it // 4 + 2)
            spt_t = spt_groups[it // 4][:, it % 4, :]
            sc = sc_pool.tile([128, H, NQ], BF16, tag="sc")
            el = el_pool.tile([128, H, NQ], BF16, tag="el")
            e2i = e2_pool.tile([128, H, NQ], I16, tag="e2")
            sps_l = []
            for hp in range(2):
                sps = ps_t.tile([128, 2, NQ], F32, tag="t")
                sps_l.append(sps)
                if it < 2:
                    keep_warm(sps[0:64, 0, 0:64], 8)
                elif it < 4:
                    keep_warm(sps[0:64, 0, 0:64], 3)
                for jj in range(2):
                    ro = jj * 64
                    nc.tensor.matmul(
                        sps[:, jj, :],
                        k_sb[it // 4][ro:ro + 64, hp,
                                      (it % 4) * 128:(it % 4) * 128 + 128],
                        q_sb[hp][ro:ro + 64, :],
                        start=True, stop=True)
                nc.scalar.copy(sc[:, 2 * hp:2 * hp + 2, :], sps[:])
            if it < NIT - 1:
                nc.vector.tensor_mul(el[:], sc[:], head_bcast(spt_t, H))
                nc.vector.tensor_scalar(e2i[:], el[:], EC1, EC2, Mul, Add)
                # messages run TWO iterations behind so the producer chain
                # (ACT copy -> DVE TT -> DVE TS) is finished when PE arrives
                if len(pendq) >= 2:
                    emit_msgs(pendq.pop(0))
                pendq.append((it, e2i))
            else:
                # final iteration: drain pending messages, then pipeline the
                # last chunk + tail over two query halves
                for p in pendq:
                    emit_msgs(p)
                pendq = []

        pst_ctx.close()

        # ---- late inputs (only needed after the attention loop) ----
        # per-head row slices of W1^T at base partition 0 (PE requires lhsT
        # and rhs to share the base partition).  All late inputs ride the
        # sync ring: its sequencer is idle by now, while a DIRECT2D on the
        # scalar ring would stall the ACT datapath right at tail start.
        w1t4 = sb.tile([64, H, 128], BF16, name="w1t4")
        nc.sync.dma_start(w1t4[:], w1t_d.rearrange("(h p) o -> p h o", p=64))
        w2t = sb.tile([128, 128], BF16, name="w2t")
        nc.sync.dma_start(w2t[:], w2t_d[:, :])
        w3t = sb.tile([128, D], BF16, name="w3t")
        nc.sync.dma_start(w3t[:], w3t_d[:, :])
        xqr = [sb.tile([128, NQ], F32, name=f"xqr{co}") for co in range(2)]
        for co in range(2):
            nc.sync.dma_start(xqr[co][:], xqr_d[co * 128:(co + 1) * 128, :])
        b1 = sb.tile([128, 1], F32, name="b1")
        b2 = sb.tile([128, 1], F32, name="b2")
        nc.sync.dma_start(b1[:], b1_d[:, :])
        nc.sync.dma_start(b2[:], b2_d[:, :])
        if has_bv:
            bv = sb.tile([128, 2], F32, name="bv")
            nc.sync.dma_start(bv[:], bv_d[:, :])
        if has_b3:
            b3 = sb.tile([128, 2], F32, name="b3")
            nc.sync.dma_start(b3[:], b3_d[:, :])

        # tail tiles
        HQ = NQ // 2
        dhs = sb.tile([1, H, NQ], BF16, name="dhs")
        msg4 = sb.tile([64, H, NQ], BF16, name="msg4")
        rbc = sb.tile([64, H, NQ], F32, name="rbc")
        h1 = sb.tile([128, NQ], BF16, name="h1")
        h2 = sb.tile([128, NQ], BF16, name="h2")
        ot = [sb.tile([128, NQ], F32, name=f"ot{co}") for co in range(2)]
        tb = sb.tile([128, NQ], F32, name="tb")

        pt_ctx = ExitStack()
        pd = pt_ctx.enter_context(tc.tile_pool(name="pd", bufs=1, space="PSUM"))
        pt = pt_ctx.enter_context(tc.tile_pool(name="pt", bufs=2, space="PSUM"))

        def drain_half(qh, sc, el, e2i, spt_t):
            ql = slice(qh * HQ, (qh + 1) * HQ)
            nc.vector.tensor_mul(el[:, :, ql], sc[:, :, ql],
                                 head_bcast(spt_t[:, ql], H))
            nc.vector.tensor_scalar(e2i[:, :, ql], el[:, :, ql], EC1, EC2,
                                    Mul, Add)
            e2b = e2i[:].bitcast(BF16)
            for h in range(H):
                nc.tensor.matmul(mps[:, h, ql], vt[7][:, 3, h, 0:65],
                                 e2b[:, h, ql],
                                 start=False, stop=True,
                                 skip_group_check=True)

        def tail_denoms(qh):
            # denominator copies for both halves go first (on ACT -- the DVE
            # is the tail bottleneck) so neither half's chain queues behind
            # the other's reciprocal/MLP work
            ql = slice(qh * HQ, (qh + 1) * HQ)
            nc.scalar.copy(dhs[:, 0:2, ql], mps[64:65, 0:2, ql])
            nc.scalar.copy(dhs[:, 2:4, ql], mps[64:65, 2:4, ql])

        def tail_norm(qh):
            # PE ones-broadcast -> reciprocal -> per-head multiply
            ql = slice(qh * HQ, (qh + 1) * HQ)
            dbb = pd.tile([64, H, HQ], F32, tag="d")
            for hp in range(2):
                nc.tensor.matmul(dbb[:, 2 * hp:2 * hp + 2, :], ones64[:],
                                 dhs[:, 2 * hp:2 * hp + 2, ql],
                                 start=True, stop=True)
            nc.vector.reciprocal_approx_fast(out=rbc[:, :, ql], in_=dbb[:])
            # normalize 2 heads per DVE op; the W1 matmul contracts each
            # head's 64 channels from its own row slice of w1t
            for co in range(2):
                nc.vector.tensor_mul(msg4[:, 2 * co:2 * co + 2, ql],
                                     mps[0:64, 2 * co:2 * co + 2, ql],
                                     rbc[:, 2 * co:2 * co + 2, ql])
            if has_bv:
                for h in range(H):
                    ro = (h % 2) * 64
                    nc.scalar.activation(msg4[:, h, ql], msg4[:, h, ql], Id,
                                         bias=bv[ro:ro + 64, h // 2:h // 2 + 1])

        def tail_mlp(qh):
            ql = slice(qh * HQ, (qh + 1) * HQ)
            u1 = pt.tile([128, HQ], F32, tag="t")
            for h in range(H):
                nc.tensor.matmul(u1[:], w1t4[:, h, :], msg4[:, h, ql],
                                 start=(h == 0), stop=(h == H - 1))
            nc.scalar.activation(h1[:, ql], u1[:], Relu, bias=b1[:, 0:1])
            u2 = pt.tile([128, HQ], F32, tag="t")
            nc.tensor.matmul(u2[:], w2t[:], h1[:, ql], start=True, stop=True)
            nc.scalar.activation(h2[:, ql], u2[:], Relu, bias=b2[:, 0:1])
            for co in range(2):
                u3 = pt.tile([128, HQ], F32, tag="t")
                nc.tensor.matmul(u3[:], w3t[:, co * 128:(co + 1) * 128],
                                 h2[:, ql], start=True, stop=True)
                if has_b3:
                    nc.scalar.activation(tb[:, ql], u3[:], Id,
                                         bias=b3[:, co:co + 1])
                    nc.vector.tensor_add(ot[co][:, ql], tb[:, ql],
                                         xqr[co][:, ql])
                else:
                    nc.vector.tensor_add(ot[co][:, ql], u3[:], xqr[co][:, ql])
                # split the 128KB output transfers over the two HWDGE rings
                # so the trailing DMA after the last compute is halved
                ring = nc.sync if co == 0 else nc.scalar
                ring.dma_start(out_d[co * 128:(co + 1) * 128, ql],
                               ot[co][:, ql])

        # last-chunk drain + tail, pipelined over the two query halves
        drain_half(0, sc, el, e2i, spt_t)
        drain_half(1, sc, el, e2i, spt_t)
        tail_denoms(0)
        tail_denoms(1)
        tail_norm(0)
        tail_norm(1)
        tail_mlp(0)
        tail_mlp(1)
        pt_ctx.close()

    nc.compile()
    return nc


def _prep_inputs(inputs):
    import ml_dtypes
    E4 = ml_dtypes.float8_e4m3
    bf = lambda a: np.ascontiguousarray(
        np.asarray(a, dtype=np.float32).astype(ml_dtypes.bfloat16))
    f8 = lambda a: np.ascontiguousarray(
        np.asarray(a, dtype=np.float32).astype(E4))
    f = lambda a: np.ascontiguousarray(np.asarray(a, dtype=np.float32))
    planar = lambda a: np.ascontiguousarray(
        np.asarray(a, np.float32).reshape(2, 128, -1).transpose(1, 0, 2))

    x32 = f(inputs["corr_feat_belief"][0])                  # [D, N]
    spT = np.asarray(inputs["spatial_compatibility"][0]).T  # [N(keys), N(q)]
    Wq, bq = f(inputs["Wq"]), f(inputs["bq"])
    Wk, bk = f(inputs["Wk"]), f(inputs["bk"])
    Wv, bv = f(inputs["Wv"]), f(inputs["bv"])
    W1, b1, g1, be1 = f(inputs["W1"]), f(inputs["b1"]), f(inputs["g1"]), f(inputs["be1"])
    W2, b2, g2, be2 = f(inputs["W2"]), f(inputs["b2"]), f(inputs["g2"]), f(inputs["be2"])
    W3, b3 = f(inputs["W3"]), f(inputs["b3"])

    scale = np.float32(1.0 / np.sqrt(DH))
    s1 = (g1 / np.sqrt(np.float32(1.0) + np.float32(1e-5))).astype(np.float32)
    s2 = (g2 / np.sqrt(np.float32(1.0) + np.float32(1e-5))).astype(np.float32)

    xpl = planar(x32)               # [128, 2, N]; channel c = p + 128*j
    x8 = f8(xpl)
    # quarter-major so each quarter DMA reads contiguous 2KB/partition lines
    x8q = np.ascontiguousarray(
        np.stack([x8[:, :, k * 1024:(k + 1) * 1024] for k in range(4)]))
    spT_bf = bf(spT)
    common = dict(
        x8=x8q,
        wq8=f8(planar(Wq.T) * WS),
        wk8=f8(planar(Wk.T) * WS),
        wv8=f8(planar(Wv.T) * WS),
        w1t=bf((W1 * s1[:, None]).T),
        w2t=bf((W2 * s2[:, None]).T),
        w3t=bf(W3.T),
        bq2=f((bq * scale).reshape(2, 128).T),
        bk2=f(bk.reshape(2, 128).T),
        bv2=f(bv.reshape(2, 128).T),
        b1f=f((s1 * b1 + be1).reshape(128, 1)),
        b2f=f((s2 * b2 + be2).reshape(128, 1)),
        b32=f(b3.reshape(2, 128).T),
    )
    in_maps = []
    for m in range(NCORES):
        sl = slice(m * NQ, (m + 1) * NQ)
        im = dict(common)
        im["xq8"] = np.ascontiguousarray(x8[:, :, sl])
        im["xqr"] = f(x32[:, sl])
        # group 4 key chunks per DMA: [8, 128, 4*NQ] with 4KB partition lines
        im["spt"] = np.ascontiguousarray(
            spT_bf[:, sl].reshape(8, 4, 128, NQ).transpose(0, 2, 1, 3)
            .reshape(8, 128, 4 * NQ))
        in_maps.append(im)
    flags = tuple(bool(np.any(b != 0)) for b in (bq, bk, bv, b3))
    return in_maps, flags


def _run(inputs, trace=False):
    from concourse.bass_utils import run_bass_kernel_spmd
    in_maps, flags = _prep_inputs(inputs)
    if flags not in _CACHE:
        _CACHE[flags] = _build(*flags)
    nc = _CACHE[flags]
    res = run_bass_kernel_spmd(nc, in_maps, core_ids=list(range(NCORES)),
                               trace=trace)
    out = np.concatenate([res.results[m]["out"] for m in range(NCORES)],
                         axis=1)[None]
    return np.ascontiguousarray(out.astype(np.float32)), res


def kernel(**inputs):
    out, _ = _run(inputs, trace=False)
    return out
